# revision 35
# baseline (speedup 1.0000x reference)
"""AGCNConv (GNN message passing) distributed Bass kernel for 8 TRN2 NeuronCores.

Reference math:
    h   = x @ W
    aew = edge_weight * sigmoid(adaptive_weight)
    deg = segment_sum(aew, row);  dis = where(deg>0, deg^-1/2, 0)
    out = segment_sum(h[col] * (dis[row]*aew*dis[col])[:,None], row)
    out = LeakyReLU(LayerNorm(out + bias))

Key identities exploited:
  * The global factor s = sigmoid(adaptive_weight) cancels in the symmetric
    normalization, so adaptive_weight is unused.
  * norm_e = dis[row]*ew*dis[col] depends only on edge data -> computed on
    the HOST in _preprocess; the device never computes degrees.

v3 design (no collectives): every core receives the FULL transposed node
matrix xT (bf16, replicated input) and redundantly computes h = x @ W for
all 40960 padded source rows, writing bf16 h to its local DRAM in two
20480-row windows (the int16 gather-index limit). Dest nodes are sharded
8 ways (5120 padded rows per core); edges are routed to their dest core
and grouped by (source window, dest block of 128), sorted by source row
within each group (gather locality), padded to 128-edge tiles with a
schedule shared across all cores (SPMD: one program, per-core tables).

The central hardware constraint (measured): DVE and GPSIMD arbitrate an
EXCLUSIVE shared SBUF port, and SWDGE gather-descriptor generation runs
on GPSIMD — any steady-state DVE work starves the gather DMA (kernel went
866us -> 560us when DVE left the loop). Hence:
  * S matrices (S[e,d] = norm_e * onehot(dest_off_e)) are built on the
    HOST and streamed via HWDGE as bf16 tiles — no DVE S-build.
  * All epilogue DVE ops are data-gated (via bias_gated) on the LAST
    gather so they cannot run mid-pass.

Device pipeline per core (emitted p1A, edgeA, p1B, edgeB so PE's in-order
queue never parks edge matmuls behind phase-1 work):
  1. h window w: xT chunks DMA'd in, 4 matmuls per [128,512] PSUM bank
     (lhsT = xT chunk, rhs = W, both bf16), one ACT copy -> bf16 staging,
     one DMA per 512 rows into h[w] DRAM.
  2. Edge pass window w: dma_gather h[col] rows (4 SWDGE queues rotating,
     32-tile chunks, 8 buffers in flight; ~2.9-3.4 ns/row isolated, ~4
     ns/row with concurrent bulk traffic), stream S tiles via HWDGE,
     PSUM-accumulate out_blk += S^T @ G per dest block (81 ns/matmul).
     Window-0 partials park in acc_sb; window-1 chains seed from them via
     an ACT SBUF->PSUM preload (start=False accumulation).
  3. Deferred epilogue per block: +bias, LayerNorm (ACT accum_out for
     var), gamma/beta, LeakyReLU via scalar_tensor_tensor max(x, 0.2x),
     DMA out.
"""

import sys

if "/opt/trn_rl_repo" not in sys.path:
    sys.path.insert(0, "/opt/trn_rl_repo")

import numpy as np
import ml_dtypes

from concourse import bacc, tile, mybir
from concourse.bass_utils import run_bass_kernel_spmd

# ---- problem constants (hardcoded per the harness contract) ----
N = 40000
E = 640000
D = 128
C = 8                # cores
NPC = 5000           # dest nodes per core
NB = 40              # dest blocks of 128 per core
NPCP = NB * 128      # 5120 padded dest rows per core
SRC_PAD = C * NPCP   # 40960 padded source rows
# gather windows (int16 index limit caps a window at 32768 rows); symmetric
# 20480/20480 measured faster than a small-A asymmetric split
WIN_A = 20480
WSZ = [WIN_A, SRC_PAD - WIN_A]
W0 = [0, WIN_A]
LN_EPS = 1e-5
LEAKY_ALPHA = 0.2

# ---- tunables ----
SKIP = frozenset()   # ablation flags: p1, g, sb, mm, epi
CH = 24              # gather chunk size in 128-edge tiles
NQ = 4               # SWDGE queues (ucode max 4)
GBUFS = 12           # gather buffers in flight (3 per queue)
XCH = 16             # node tiles per xT chunk DMA (2048 nodes)
PST = 4              # node tiles per P1 PSUM group ([128,512] bank)

_f32 = mybir.dt.float32
_bf16 = mybir.dt.bfloat16
_i16 = mybir.dt.int16


def _preprocess(edge_index, edge_weight):
    """Host: symmetric normalization, edge routing/grouping, shared padded
    schedule, per-core tile-layout tables (ewt/doft/idx16)."""
    row = np.asarray(edge_index[0], dtype=np.int64)
    col = np.asarray(edge_index[1], dtype=np.int64)
    ew = np.asarray(edge_weight, dtype=np.float32)
    deg = np.bincount(row, weights=ew, minlength=N)
    dis = np.where(deg > 0, deg ** -0.5, 0.0)
    ew = (dis[row] * ew * dis[col]).astype(np.float32)

    core = row // NPC
    lid = row - core * NPC
    blk = lid >> 7
    off = (lid & 127).astype(np.float32)
    win = (col >= WIN_A).astype(np.int64)     # source window 0|1
    crel = (col - win * WIN_A).astype(np.int16)  # 0..32767

    # group key per edge: (core, window, block)
    key = (core * 2 + win) * NB + blk
    counts = np.bincount(key, minlength=C * 2 * NB).reshape(C, 2, NB)
    # shared schedule: tiles per (window, block) = max over cores, rounded up
    P = 128 * np.ceil(counts.max(axis=0) / 128.0).astype(np.int64)  # [2, NB]
    for b in range(NB):
        if P[0, b] + P[1, b] == 0:
            P[1, b] = 128  # at least one (all-dummy) tile per block

    Ppad = int(P.sum())
    T = Ppad // 128
    gstart = np.concatenate([[0], np.cumsum(P.reshape(-1))])[:-1].reshape(2, NB)

    colr_a = np.zeros((C, Ppad), np.int16)
    ew_a = np.zeros((C, Ppad), np.float32)
    dof_a = np.zeros((C, Ppad), np.float32)

    # sort by (group, source row): order within a group is free, ascending
    # source addresses give the gather slightly better DRAM locality
    order = np.lexsort((crel, key))
    key_s = key[order]
    grp_sizes = counts.reshape(-1)
    grp_off = np.concatenate([[0], np.cumsum(grp_sizes)])[:-1]
    within = np.arange(len(key_s)) - grp_off[key_s]
    c_s = key_s // (2 * NB)
    wb_s = key_s % (2 * NB)
    dest = gstart.reshape(-1)[wb_s] + within
    colr_a[c_s, dest] = crel[order]
    ew_a[c_s, dest] = ew[order]
    dof_a[c_s, dest] = off[order]

    # S tiles precomputed on HOST (static edge data): S[e, d] = norm_e if
    # dest_off_e == d else 0.  Layout [C, 128 edge, T*128] bf16 so tile t is
    # the column slice [:, 128t:128(t+1)] — streamed to SBUF and fed to the
    # PE as lhsT directly.  This keeps DVE silent during the edge pass: DVE
    # work would lock GPSIMD out of the shared SBUF port it needs to write
    # SWDGE gather descriptors, starving the gather DMA.
    st = np.zeros((C, 128, T * 128), ml_dtypes.bfloat16)
    cc = c_s
    tt = dest // 128
    ee = (dest % 128).astype(np.int64)
    dd = dof_a[c_s, dest].astype(np.int64)
    st[cc, ee, tt * 128 + dd] = ew[order].astype(ml_dtypes.bfloat16)
    # dma_gather index layout: idx i of a tile at [i%16, i//16], replicated x8
    A = colr_a.reshape(C, T, 8, 16).transpose(0, 1, 3, 2)                      # [C,T,16,8]
    idx16 = np.tile(A, (1, 1, 8, 1)).transpose(0, 2, 1, 3).reshape(C, 128, 8 * T)
    return P, T, st, np.ascontiguousarray(idx16)


def _schedule(P):
    """Static tile schedule shared by all cores."""
    P = np.asarray(P)
    tiles = []  # global tile idx -> (w, b)
    for w in (0, 1):
        for b in range(NB):
            for _ in range(int(P[w, b]) // 128):
                tiles.append((w, b))
    T = len(tiles)
    t0w = [0, int(P[0].sum()) // 128]
    Tw = [t0w[1], T - t0w[1]]
    return tiles, t0w, Tw


def _build(P, T, tiles, t0w, Tw, sim_single_core=False, reps=1):
    del sim_single_core  # v2 has no collectives; kept for test.py compat
    nc = bacc.Bacc("TRN2", target_bir_lowering=False, debug=False,
                   enable_asserts=True, num_devices=1,
                   num_swdge_queues=NQ, dynamic_dma_scratch_size=65536)

    xt_in = nc.dram_tensor("xt", [128, SRC_PAD], _bf16, kind="ExternalInput").ap()
    w_in = nc.dram_tensor("w", [D, D], _bf16, kind="ExternalInput").ap()
    bias_in = nc.dram_tensor("bias", [1, D], _f32, kind="ExternalInput").ap()
    gamma_in = nc.dram_tensor("gamma", [1, D], _f32, kind="ExternalInput").ap()
    beta_in = nc.dram_tensor("beta", [1, D], _f32, kind="ExternalInput").ap()
    st_in = nc.dram_tensor("st", [128, 128 * T], _bf16, kind="ExternalInput").ap()
    idx_in = nc.dram_tensor("idx16", [128, 8 * T], _i16, kind="ExternalInput").ap()
    out_d = nc.dram_tensor("out", [NPCP, D], _f32, kind="ExternalOutput").ap()

    eq = mybir.AluOpType.is_equal
    mul = mybir.AluOpType.mult
    add = mybir.AluOpType.add
    AF = mybir.ActivationFunctionType

    # per-block window runs: (first_tile, last_tile) or None
    runs = [[None, None] for _ in range(NB)]
    for t, (w, b) in enumerate(tiles):
        if runs[b][w] is None:
            runs[b][w] = [t, t]
        else:
            runs[b][w][1] = t

    with tile.TileContext(nc) as tc:
        with (
            tc.tile_pool(name="const", bufs=1) as cp,
            tc.tile_pool(name="resident", bufs=1) as rp,
            tc.tile_pool(name="dram", bufs=1, space="DRAM") as dp,
        ):
            ones_row = cp.tile([1, 128], _f32)
            nc.vector.memset(ones_row[:], 1.0)
            eps_col = cp.tile([128, 1], _f32)
            nc.vector.memset(eps_col[:], float(LN_EPS))
            w_sb = cp.tile([128, 128], _bf16)
            nc.sync.dma_start(w_sb[:], w_in)

            # broadcast bias/gamma/beta rows to all 128 partitions via matmul
            bias_bc = cp.tile([128, 128], _f32)
            gamma_bc = cp.tile([128, 128], _f32)
            beta_bc = cp.tile([128, 128], _f32)
            with tc.tile_pool(name="bc", bufs=1) as bcp, \
                 tc.tile_pool(name="bcps", bufs=1, space="PSUM") as bcps:
                for src_ap, dst in ((bias_in, bias_bc), (gamma_in, gamma_bc),
                                    (beta_in, beta_bc)):
                    r = bcp.tile([1, 128], _f32, tag="bcrow")
                    nc.sync.dma_start(r[:], src_ap)
                    ps = bcps.tile([128, 128], _f32, tag="bcps")
                    nc.tensor.matmul(ps[:], lhsT=ones_row[:], rhs=r[:],
                                     start=True, stop=True)
                    nc.scalar.copy(dst[:], ps[:])

            idx_sb = rp.tile([128, 8 * T], _i16)
            nc.sync.dma_start(idx_sb[:], idx_in)

            acc_sb = rp.tile([128, NB, 128], _f32)  # window-0 partial sums

            h_a = dp.tile([WSZ[0], D], _bf16)
            h_b = dp.tile([WSZ[1], D], _bf16)
            h_d = [h_a, h_b]

            dmaeng = [nc.sync, nc.scalar]  # HWDGE streams for plain DMAs

            def _phases():
                # ---------------- Phase 1: h[w] = bf16(x @ W) ----------------
                def p1_window(w):
                    if "p1" in SKIP:
                        # ablation: fill h with arbitrary bytes in one DMA so
                        # gather reads aren't reads-before-any-write
                        hv = h_d[w][:].rearrange("(t p) d -> p t d", p=128)
                        src = xt_in[:, W0[w]:W0[w] + WSZ[w]].rearrange(
                            "p (t d) -> p t d", d=128)
                        nc.sync.dma_start(hv, src)
                        return
                    nchunks = WSZ[w] // (XCH * 128)
                    for ci in range(nchunks):
                        base = W0[w] + ci * XCH * 128
                        xc = p1x.tile([128, XCH * 128], _bf16, tag="xc")
                        dmaeng[ci % 2].dma_start(
                            xc[:], xt_in[:, base:base + XCH * 128])
                        for g in range(XCH // PST):
                            ps = p1ps.tile([128, PST * 128], _f32, tag="hps")
                            for j in range(PST):
                                k = (g * PST + j) * 128
                                nc.tensor.matmul(
                                    ps[:, j * 128:(j + 1) * 128],
                                    lhsT=xc[:, k:k + 128], rhs=w_sb[:],
                                    start=True, stop=True)
                            hst = p1h.tile([128, PST, 128], _bf16, tag="hst")
                            nc.scalar.copy(hst[:], ps[:])
                            r0 = ci * XCH * 128 + g * PST * 128
                            dst = h_d[w][r0:r0 + PST * 128, :].rearrange(
                                "(t p) d -> p t d", p=128)
                            dmaeng[(ci + g) % 2].dma_start(dst, hst[:])

                # ---------------- Edge pass: gather + scatter matmuls --------
                def epilogue(b, bias_src):
                    # gated on bias_src (copied only after the final gather):
                    # epilogue DVE ops would otherwise run mid-pass and lock
                    # GPSIMD out of SWDGE gather-descriptor generation
                    if "epi" in SKIP:
                        return
                    t4 = ep.tile([128, 128], _f32, tag="e_t4")
                    nc.vector.tensor_tensor(t4[:], acc_sb[:, b, :],
                                            bias_src[:], op=add)
                    nsum = ep.tile([128, 1], _f32, tag="e_ns")
                    nc.vector.tensor_reduce(nsum[:], t4[:],
                                            axis=mybir.AxisListType.X,
                                            op=add, negate=True)
                    nmean = ep.tile([128, 1], _f32, tag="e_nm")
                    nc.scalar.mul(nmean[:], nsum[:], 1.0 / 128.0)
                    t5 = ep.tile([128, 128], _f32, tag="e_t5")
                    nc.scalar.activation(t5[:], t4[:], AF.Identity,
                                         bias=nmean[:], scale=1.0)
                    sq = ep.tile([128, 128], _f32, tag="e_sq")
                    vsum = ep.tile([128, 1], _f32, tag="e_vs")
                    nc.scalar.activation(sq[:], t5[:], AF.Square,
                                         accum_out=vsum[:])
                    sd = ep.tile([128, 1], _f32, tag="e_sd")
                    nc.scalar.activation(sd[:], vsum[:], AF.Sqrt,
                                         scale=1.0 / 128.0, bias=eps_col[:])
                    rstd = ep.tile([128, 1], _f32, tag="e_rs")
                    nc.vector.reciprocal(rstd[:], sd[:])
                    t6 = ep.tile([128, 128], _f32, tag="e_t6")
                    nc.vector.scalar_tensor_tensor(t6[:], t5[:], rstd[:],
                                                   gamma_bc[:], op0=mul, op1=mul)
                    nc.vector.tensor_tensor(t6[:], t6[:], beta_bc[:], op=add)
                    osb = ep.tile([128, 128], _f32, tag="e_o")
                    nc.vector.scalar_tensor_tensor(osb[:], t6[:],
                                                   float(LEAKY_ALPHA), t6[:],
                                                   op0=mul,
                                                   op1=mybir.AluOpType.max)
                    nc.sync.dma_start(out_d[b * 128:(b + 1) * 128, :], osb[:])

                gcall = [0]

                def edge_window(w):
                    win_ap = h_d[w][:]
                    nt_left = Tw[w]
                    t0 = t0w[w]
                    cur_ps = None
                    while nt_left > 0:
                        nt = min(CH, nt_left)
                        gbuf = gbp.tile([128, CH, 128], _bf16, tag="g")
                        if "gs" in SKIP:
                            # ablation: same bytes, sequential HWDGE DMA
                            src = h_d[w][:nt * 128, :].rearrange(
                                "(t p) d -> p t d", p=128)
                            dmaeng[gcall[0] % 2].dma_start(gbuf[:, :nt, :], src)
                            gcall[0] += 1
                        elif "g" in SKIP:
                            nc.vector.memset(gbuf[:], 0.0)
                        else:
                            nc.gpsimd.dma_gather(
                                out_ap=gbuf[:, :nt, :], in_ap=win_ap,
                                idxs_ap=idx_sb[:, 8 * t0:8 * (t0 + nt)],
                                num_idxs=128 * nt, num_idxs_reg=128 * nt,
                                elem_size=128, single_packet=False,
                                queue_num=gcall[0] % NQ)
                            gcall[0] += 1
                        if "sb" not in SKIP:  # "sb" skip implies "mm" skip
                            st_sb = stp.tile([128, CH * 128], _bf16, tag="st")
                            dmaeng[gcall[0] % 2].dma_start(
                                st_sb[:, :nt * 128],
                                st_in[:, 128 * t0:128 * (t0 + nt)])
                        for s_i in range(nt):
                            t = t0 + s_i
                            b = tiles[t][1]
                            first = runs[b][w][0] == t
                            last = runs[b][w][1] == t
                            preload = w == 1 and runs[b][0] is not None
                            if first:
                                cur_ps = pbps.tile([128, 128], _f32, tag="blk")
                                if preload and "mm" not in SKIP:
                                    # seed the accumulation with the window-0
                                    # partial so no separate add is needed
                                    nc.scalar.copy(cur_ps[:], acc_sb[:, b, :])
                                if "mm" in SKIP:
                                    nc.vector.memset(cur_ps[:], 0.0)
                            if "mm" not in SKIP:
                                nc.tensor.matmul(
                                    cur_ps[:],
                                    lhsT=st_sb[:, 128 * s_i:128 * (s_i + 1)],
                                    rhs=gbuf[:, s_i, :],
                                    start=first and not preload, stop=last)
                            if last:
                                nc.scalar.copy(acc_sb[:, b, :], cur_ps[:])
                                cur_ps = None
                        t0 += nt
                        nt_left -= nt

                with tc.tile_pool(name="p1x", bufs=2) as p1x, \
                     tc.tile_pool(name="p1h", bufs=3) as p1h, \
                     tc.tile_pool(name="p1ps", bufs=3, space="PSUM") as p1ps, \
                     tc.tile_pool(name="stp", bufs=3) as stp, \
                     tc.tile_pool(name="gb", bufs=GBUFS) as gbp, \
                     tc.tile_pool(name="pbps", bufs=4, space="PSUM") as pbps, \
                     tc.tile_pool(name="ep", bufs=2) as ep:
                    # p1(0), edge(0), p1(1), edge(1): keeps PE's in-order
                    # queue from parking edge-A matmuls behind P1-B, which
                    # would stall gather-A once the gather buffers fill
                    p1_window(0)
                    edge_window(0)
                    p1_window(1)
                    edge_window(1)
                    # bias_gated is written after the last edge-pass psum
                    # copy in ACT program order; every epilogue reads it, so
                    # no epilogue DVE op can start before the gathers end
                    bias_gated = ep.tile([128, 128], _f32, tag="e_bg")
                    nc.scalar.copy(bias_gated[:], bias_bc[:])
                    for b in range(NB):
                        epilogue(b, bias_gated)

            if reps == 1:
                _phases()
            else:
                with tc.For_i(0, reps, 1):
                    _phases()

    nc.compile()
    return nc


def _core_maps(x, weight, bias, gamma, beta, st, idx16):
    """Per-core input maps (shared by kernel() and test.py)."""
    xt = np.zeros((128, SRC_PAD), ml_dtypes.bfloat16)
    xt[:, :N] = np.asarray(x, np.float32).T.astype(ml_dtypes.bfloat16)
    wb = np.asarray(weight, np.float32).astype(ml_dtypes.bfloat16)
    bias = np.asarray(bias, np.float32).reshape(1, D)
    gamma = np.asarray(gamma, np.float32).reshape(1, D)
    beta = np.asarray(beta, np.float32).reshape(1, D)
    maps = []
    for c in range(C):
        maps.append({
            "xt": xt, "w": wb, "bias": bias, "gamma": gamma, "beta": beta,
            "st": np.ascontiguousarray(st[c]),
            "idx16": np.ascontiguousarray(idx16[c]),
        })
    return maps


_CACHE = {}


def _get_compiled(edge_index, edge_weight):
    P, T, st, idx16 = _preprocess(edge_index, edge_weight)
    key = P.tobytes()
    if key not in _CACHE:
        tiles, t0w, Tw = _schedule(P)
        _CACHE[key] = _build(P, T, tiles, t0w, Tw)
    return _CACHE[key], st, idx16


def kernel(x, edge_index, edge_weight, weight, adaptive_weight, bias,
           ln_gamma, ln_beta):
    nc, st, idx16 = _get_compiled(edge_index, edge_weight)
    in_maps = _core_maps(x, weight, bias, ln_gamma, ln_beta, st, idx16)
    res = run_bass_kernel_spmd(nc, in_maps, core_ids=list(range(C)))
    out = np.empty((N, D), np.float32)
    for c in range(C):
        out[c * NPC:(c + 1) * NPC] = res.results[c]["out"][:NPC]
    return out


# revision 36
# speedup vs baseline: 1.0183x; 1.0183x over previous
"""AGCNConv (GNN message passing) distributed Bass kernel for 8 TRN2 NeuronCores.

Reference math:
    h   = x @ W
    aew = edge_weight * sigmoid(adaptive_weight)
    deg = segment_sum(aew, row);  dis = where(deg>0, deg^-1/2, 0)
    out = segment_sum(h[col] * (dis[row]*aew*dis[col])[:,None], row)
    out = LeakyReLU(LayerNorm(out + bias))

Key identities exploited:
  * The global factor s = sigmoid(adaptive_weight) cancels in the symmetric
    normalization, so adaptive_weight is unused.
  * norm_e = dis[row]*ew*dis[col] depends only on edge data -> computed on
    the HOST in _preprocess; the device never computes degrees.

v3 design (no collectives): every core receives the FULL transposed node
matrix xT (bf16, replicated input) and redundantly computes h = x @ W for
all 40960 padded source rows, writing bf16 h to its local DRAM in two
20480-row windows (the int16 gather-index limit). Dest nodes are sharded
8 ways (5120 padded rows per core); edges are routed to their dest core
and grouped by (source window, dest block of 128), sorted by source row
within each group (gather locality), padded to 128-edge tiles with a
schedule shared across all cores (SPMD: one program, per-core tables).

The central hardware constraint (measured): DVE and GPSIMD arbitrate an
EXCLUSIVE shared SBUF port, and SWDGE gather-descriptor generation runs
on GPSIMD — any steady-state DVE work starves the gather DMA (kernel went
866us -> 560us when DVE left the loop). Hence:
  * S matrices (S[e,d] = norm_e * onehot(dest_off_e)) are built on the
    HOST and streamed via HWDGE as bf16 tiles — no DVE S-build.
  * All epilogue DVE ops are data-gated (via bias_gated) on the LAST
    gather so they cannot run mid-pass.

Device pipeline per core (emitted p1A, edgeA, p1B, edgeB so PE's in-order
queue never parks edge matmuls behind phase-1 work):
  1. h window w: xT chunks DMA'd in, 4 matmuls per [128,512] PSUM bank
     (lhsT = xT chunk, rhs = W, both bf16), one ACT copy -> bf16 staging,
     one DMA per 512 rows into h[w] DRAM.
  2. Edge pass window w: dma_gather h[col] rows (4 SWDGE queues rotating,
     32-tile chunks, 8 buffers in flight; ~2.9-3.4 ns/row isolated, ~4
     ns/row with concurrent bulk traffic), stream S tiles via HWDGE,
     PSUM-accumulate out_blk += S^T @ G per dest block (81 ns/matmul).
     Window-0 partials park in acc_sb; window-1 chains seed from them via
     an ACT SBUF->PSUM preload (start=False accumulation).
  3. Deferred epilogue per block: +bias, LayerNorm (ACT accum_out for
     var), gamma/beta, LeakyReLU via scalar_tensor_tensor max(x, 0.2x),
     DMA out.
"""

import sys

if "/opt/trn_rl_repo" not in sys.path:
    sys.path.insert(0, "/opt/trn_rl_repo")

import numpy as np
import ml_dtypes

from concourse import bacc, tile, mybir
from concourse.bass_utils import run_bass_kernel_spmd

# ---- problem constants (hardcoded per the harness contract) ----
N = 40000
E = 640000
D = 128
C = 8                # cores
NPC = 5000           # dest nodes per core
NB = 40              # dest blocks of 128 per core
NPCP = NB * 128      # 5120 padded dest rows per core
SRC_PAD = C * NPCP   # 40960 padded source rows
# gather windows (int16 index limit caps a window at 32768 rows); symmetric
# 20480/20480 measured faster than a small-A asymmetric split
WIN_A = 20480
WSZ = [WIN_A, SRC_PAD - WIN_A]
W0 = [0, WIN_A]
LN_EPS = 1e-5
LEAKY_ALPHA = 0.2

# ---- tunables ----
SKIP = frozenset()   # ablation flags: p1, g, sb, mm, epi
CH = 32              # gather chunk size in 128-edge tiles (24/12 and 64/4
NQ = 4               # variants measured slower; 32/8 is the sweet spot)
GBUFS = 8            # gather buffers in flight (2 per queue)
XCH = 16             # node tiles per xT chunk DMA (2048 nodes)
PST = 4              # node tiles per P1 PSUM group ([128,512] bank)

_f32 = mybir.dt.float32
_bf16 = mybir.dt.bfloat16
_i16 = mybir.dt.int16


def _preprocess(edge_index, edge_weight):
    """Host: symmetric normalization, edge routing/grouping, shared padded
    schedule, per-core tile-layout tables (ewt/doft/idx16)."""
    row = np.asarray(edge_index[0], dtype=np.int64)
    col = np.asarray(edge_index[1], dtype=np.int64)
    ew = np.asarray(edge_weight, dtype=np.float32)
    deg = np.bincount(row, weights=ew, minlength=N)
    dis = np.where(deg > 0, deg ** -0.5, 0.0)
    ew = (dis[row] * ew * dis[col]).astype(np.float32)

    core = row // NPC
    lid = row - core * NPC
    blk = lid >> 7
    off = (lid & 127).astype(np.float32)
    win = (col >= WIN_A).astype(np.int64)     # source window 0|1
    crel = (col - win * WIN_A).astype(np.int16)  # 0..32767

    # group key per edge: (core, window, block)
    key = (core * 2 + win) * NB + blk
    counts = np.bincount(key, minlength=C * 2 * NB).reshape(C, 2, NB)
    # shared schedule: tiles per (window, block) = max over cores, rounded up
    P = 128 * np.ceil(counts.max(axis=0) / 128.0).astype(np.int64)  # [2, NB]
    for b in range(NB):
        if P[0, b] + P[1, b] == 0:
            P[1, b] = 128  # at least one (all-dummy) tile per block

    Ppad = int(P.sum())
    T = Ppad // 128
    gstart = np.concatenate([[0], np.cumsum(P.reshape(-1))])[:-1].reshape(2, NB)

    colr_a = np.zeros((C, Ppad), np.int16)
    ew_a = np.zeros((C, Ppad), np.float32)
    dof_a = np.zeros((C, Ppad), np.float32)

    # sort by (group, source row): order within a group is free, ascending
    # source addresses give the gather slightly better DRAM locality
    order = np.lexsort((crel, key))
    key_s = key[order]
    grp_sizes = counts.reshape(-1)
    grp_off = np.concatenate([[0], np.cumsum(grp_sizes)])[:-1]
    within = np.arange(len(key_s)) - grp_off[key_s]
    c_s = key_s // (2 * NB)
    wb_s = key_s % (2 * NB)
    dest = gstart.reshape(-1)[wb_s] + within
    colr_a[c_s, dest] = crel[order]
    ew_a[c_s, dest] = ew[order]
    dof_a[c_s, dest] = off[order]

    # S tiles precomputed on HOST (static edge data): S[e, d] = norm_e if
    # dest_off_e == d else 0.  Layout [C, 128 edge, T*128] bf16 so tile t is
    # the column slice [:, 128t:128(t+1)] — streamed to SBUF and fed to the
    # PE as lhsT directly.  This keeps DVE silent during the edge pass: DVE
    # work would lock GPSIMD out of the shared SBUF port it needs to write
    # SWDGE gather descriptors, starving the gather DMA.
    st = np.zeros((C, 128, T * 128), ml_dtypes.bfloat16)
    cc = c_s
    tt = dest // 128
    ee = (dest % 128).astype(np.int64)
    dd = dof_a[c_s, dest].astype(np.int64)
    st[cc, ee, tt * 128 + dd] = ew[order].astype(ml_dtypes.bfloat16)
    # dma_gather index layout: idx i of a tile at [i%16, i//16], replicated x8
    A = colr_a.reshape(C, T, 8, 16).transpose(0, 1, 3, 2)                      # [C,T,16,8]
    idx16 = np.tile(A, (1, 1, 8, 1)).transpose(0, 2, 1, 3).reshape(C, 128, 8 * T)
    return P, T, st, np.ascontiguousarray(idx16)


def _schedule(P):
    """Static tile schedule shared by all cores."""
    P = np.asarray(P)
    tiles = []  # global tile idx -> (w, b)
    for w in (0, 1):
        for b in range(NB):
            for _ in range(int(P[w, b]) // 128):
                tiles.append((w, b))
    T = len(tiles)
    t0w = [0, int(P[0].sum()) // 128]
    Tw = [t0w[1], T - t0w[1]]
    return tiles, t0w, Tw


def _build(P, T, tiles, t0w, Tw, sim_single_core=False, reps=1):
    del sim_single_core  # v2 has no collectives; kept for test.py compat
    nc = bacc.Bacc("TRN2", target_bir_lowering=False, debug=False,
                   enable_asserts=True, num_devices=1,
                   num_swdge_queues=NQ, dynamic_dma_scratch_size=65536)

    xt_in = nc.dram_tensor("xt", [128, SRC_PAD], _bf16, kind="ExternalInput").ap()
    w_in = nc.dram_tensor("w", [D, D], _bf16, kind="ExternalInput").ap()
    bias_in = nc.dram_tensor("bias", [1, D], _f32, kind="ExternalInput").ap()
    gamma_in = nc.dram_tensor("gamma", [1, D], _f32, kind="ExternalInput").ap()
    beta_in = nc.dram_tensor("beta", [1, D], _f32, kind="ExternalInput").ap()
    st_in = nc.dram_tensor("st", [128, 128 * T], _bf16, kind="ExternalInput").ap()
    idx_in = nc.dram_tensor("idx16", [128, 8 * T], _i16, kind="ExternalInput").ap()
    out_d = nc.dram_tensor("out", [NPCP, D], _f32, kind="ExternalOutput").ap()

    eq = mybir.AluOpType.is_equal
    mul = mybir.AluOpType.mult
    add = mybir.AluOpType.add
    AF = mybir.ActivationFunctionType

    # per-block window runs: (first_tile, last_tile) or None
    runs = [[None, None] for _ in range(NB)]
    for t, (w, b) in enumerate(tiles):
        if runs[b][w] is None:
            runs[b][w] = [t, t]
        else:
            runs[b][w][1] = t

    with tile.TileContext(nc) as tc:
        with (
            tc.tile_pool(name="const", bufs=1) as cp,
            tc.tile_pool(name="resident", bufs=1) as rp,
            tc.tile_pool(name="dram", bufs=1, space="DRAM") as dp,
        ):
            ones_row = cp.tile([1, 128], _f32)
            nc.vector.memset(ones_row[:], 1.0)
            eps_col = cp.tile([128, 1], _f32)
            nc.vector.memset(eps_col[:], float(LN_EPS))
            w_sb = cp.tile([128, 128], _bf16)
            nc.sync.dma_start(w_sb[:], w_in)

            # broadcast bias/gamma/beta rows to all 128 partitions via matmul
            bias_bc = cp.tile([128, 128], _f32)
            gamma_bc = cp.tile([128, 128], _f32)
            beta_bc = cp.tile([128, 128], _f32)
            with tc.tile_pool(name="bc", bufs=1) as bcp, \
                 tc.tile_pool(name="bcps", bufs=1, space="PSUM") as bcps:
                for src_ap, dst in ((bias_in, bias_bc), (gamma_in, gamma_bc),
                                    (beta_in, beta_bc)):
                    r = bcp.tile([1, 128], _f32, tag="bcrow")
                    nc.sync.dma_start(r[:], src_ap)
                    ps = bcps.tile([128, 128], _f32, tag="bcps")
                    nc.tensor.matmul(ps[:], lhsT=ones_row[:], rhs=r[:],
                                     start=True, stop=True)
                    nc.scalar.copy(dst[:], ps[:])

            idx_sb = rp.tile([128, 8 * T], _i16)
            nc.sync.dma_start(idx_sb[:], idx_in)

            acc_sb = rp.tile([128, NB, 128], _f32)  # window-0 partial sums

            h_a = dp.tile([WSZ[0], D], _bf16)
            h_b = dp.tile([WSZ[1], D], _bf16)
            h_d = [h_a, h_b]

            dmaeng = [nc.sync, nc.scalar]  # HWDGE streams for plain DMAs

            def _phases():
                # ---------------- Phase 1: h[w] = bf16(x @ W) ----------------
                def p1_window(w):
                    if "p1" in SKIP:
                        # ablation: fill h with arbitrary bytes in one DMA so
                        # gather reads aren't reads-before-any-write
                        hv = h_d[w][:].rearrange("(t p) d -> p t d", p=128)
                        src = xt_in[:, W0[w]:W0[w] + WSZ[w]].rearrange(
                            "p (t d) -> p t d", d=128)
                        nc.sync.dma_start(hv, src)
                        return
                    nchunks = WSZ[w] // (XCH * 128)
                    for ci in range(nchunks):
                        base = W0[w] + ci * XCH * 128
                        xc = p1x.tile([128, XCH * 128], _bf16, tag="xc")
                        dmaeng[ci % 2].dma_start(
                            xc[:], xt_in[:, base:base + XCH * 128])
                        for g in range(XCH // PST):
                            ps = p1ps.tile([128, PST * 128], _f32, tag="hps")
                            for j in range(PST):
                                k = (g * PST + j) * 128
                                nc.tensor.matmul(
                                    ps[:, j * 128:(j + 1) * 128],
                                    lhsT=xc[:, k:k + 128], rhs=w_sb[:],
                                    start=True, stop=True)
                            hst = p1h.tile([128, PST, 128], _bf16, tag="hst")
                            nc.scalar.copy(hst[:], ps[:])
                            r0 = ci * XCH * 128 + g * PST * 128
                            dst = h_d[w][r0:r0 + PST * 128, :].rearrange(
                                "(t p) d -> p t d", p=128)
                            dmaeng[(ci + g) % 2].dma_start(dst, hst[:])

                # ---------------- Edge pass: gather + scatter matmuls --------
                def epilogue(b, bias_src):
                    # gated on bias_src (copied only after the final gather):
                    # epilogue DVE ops would otherwise run mid-pass and lock
                    # GPSIMD out of SWDGE gather-descriptor generation
                    if "epi" in SKIP:
                        return
                    t4 = ep.tile([128, 128], _f32, tag="e_t4")
                    nc.vector.tensor_tensor(t4[:], acc_sb[:, b, :],
                                            bias_src[:], op=add)
                    nsum = ep.tile([128, 1], _f32, tag="e_ns")
                    nc.vector.tensor_reduce(nsum[:], t4[:],
                                            axis=mybir.AxisListType.X,
                                            op=add, negate=True)
                    nmean = ep.tile([128, 1], _f32, tag="e_nm")
                    nc.scalar.mul(nmean[:], nsum[:], 1.0 / 128.0)
                    t5 = ep.tile([128, 128], _f32, tag="e_t5")
                    nc.scalar.activation(t5[:], t4[:], AF.Identity,
                                         bias=nmean[:], scale=1.0)
                    sq = ep.tile([128, 128], _f32, tag="e_sq")
                    vsum = ep.tile([128, 1], _f32, tag="e_vs")
                    nc.scalar.activation(sq[:], t5[:], AF.Square,
                                         accum_out=vsum[:])
                    sd = ep.tile([128, 1], _f32, tag="e_sd")
                    nc.scalar.activation(sd[:], vsum[:], AF.Sqrt,
                                         scale=1.0 / 128.0, bias=eps_col[:])
                    rstd = ep.tile([128, 1], _f32, tag="e_rs")
                    nc.vector.reciprocal(rstd[:], sd[:])
                    t6 = ep.tile([128, 128], _f32, tag="e_t6")
                    nc.vector.scalar_tensor_tensor(t6[:], t5[:], rstd[:],
                                                   gamma_bc[:], op0=mul, op1=mul)
                    nc.vector.tensor_tensor(t6[:], t6[:], beta_bc[:], op=add)
                    osb = ep.tile([128, 128], _f32, tag="e_o")
                    nc.vector.scalar_tensor_tensor(osb[:], t6[:],
                                                   float(LEAKY_ALPHA), t6[:],
                                                   op0=mul,
                                                   op1=mybir.AluOpType.max)
                    nc.sync.dma_start(out_d[b * 128:(b + 1) * 128, :], osb[:])

                gcall = [0]

                def edge_window(w):
                    win_ap = h_d[w][:]
                    nt_left = Tw[w]
                    t0 = t0w[w]
                    cur_ps = None
                    while nt_left > 0:
                        nt = min(CH, nt_left)
                        gbuf = gbp.tile([128, CH, 128], _bf16, tag="g")
                        if "gs" in SKIP:
                            # ablation: same bytes, sequential HWDGE DMA
                            src = h_d[w][:nt * 128, :].rearrange(
                                "(t p) d -> p t d", p=128)
                            dmaeng[gcall[0] % 2].dma_start(gbuf[:, :nt, :], src)
                            gcall[0] += 1
                        elif "g" in SKIP:
                            nc.vector.memset(gbuf[:], 0.0)
                        else:
                            nc.gpsimd.dma_gather(
                                out_ap=gbuf[:, :nt, :], in_ap=win_ap,
                                idxs_ap=idx_sb[:, 8 * t0:8 * (t0 + nt)],
                                num_idxs=128 * nt, num_idxs_reg=128 * nt,
                                elem_size=128, single_packet=False,
                                queue_num=gcall[0] % NQ)
                            gcall[0] += 1
                        if "sb" not in SKIP:  # "sb" skip implies "mm" skip
                            st_sb = stp.tile([128, CH * 128], _bf16, tag="st")
                            dmaeng[gcall[0] % 2].dma_start(
                                st_sb[:, :nt * 128],
                                st_in[:, 128 * t0:128 * (t0 + nt)])
                        for s_i in range(nt):
                            t = t0 + s_i
                            b = tiles[t][1]
                            first = runs[b][w][0] == t
                            last = runs[b][w][1] == t
                            preload = w == 1 and runs[b][0] is not None
                            if first:
                                cur_ps = pbps.tile([128, 128], _f32, tag="blk")
                                if preload and "mm" not in SKIP:
                                    # seed the accumulation with the window-0
                                    # partial so no separate add is needed
                                    nc.scalar.copy(cur_ps[:], acc_sb[:, b, :])
                                if "mm" in SKIP:
                                    nc.vector.memset(cur_ps[:], 0.0)
                            if "mm" not in SKIP:
                                nc.tensor.matmul(
                                    cur_ps[:],
                                    lhsT=st_sb[:, 128 * s_i:128 * (s_i + 1)],
                                    rhs=gbuf[:, s_i, :],
                                    start=first and not preload, stop=last)
                            if last:
                                nc.scalar.copy(acc_sb[:, b, :], cur_ps[:])
                                cur_ps = None
                        t0 += nt
                        nt_left -= nt

                with tc.tile_pool(name="p1x", bufs=2) as p1x, \
                     tc.tile_pool(name="p1h", bufs=3) as p1h, \
                     tc.tile_pool(name="p1ps", bufs=3, space="PSUM") as p1ps, \
                     tc.tile_pool(name="stp", bufs=3) as stp, \
                     tc.tile_pool(name="gb", bufs=GBUFS) as gbp, \
                     tc.tile_pool(name="pbps", bufs=4, space="PSUM") as pbps, \
                     tc.tile_pool(name="ep", bufs=2) as ep:
                    # p1(0), edge(0), p1(1), edge(1): keeps PE's in-order
                    # queue from parking edge-A matmuls behind P1-B, which
                    # would stall gather-A once the gather buffers fill
                    p1_window(0)
                    edge_window(0)
                    p1_window(1)
                    edge_window(1)
                    # bias_gated is written after the last edge-pass psum
                    # copy in ACT program order; every epilogue reads it, so
                    # no epilogue DVE op can start before the gathers end
                    bias_gated = ep.tile([128, 128], _f32, tag="e_bg")
                    nc.scalar.copy(bias_gated[:], bias_bc[:])
                    for b in range(NB):
                        epilogue(b, bias_gated)

            if reps == 1:
                _phases()
            else:
                with tc.For_i(0, reps, 1):
                    _phases()

    nc.compile()
    return nc


def _core_maps(x, weight, bias, gamma, beta, st, idx16):
    """Per-core input maps (shared by kernel() and test.py)."""
    xt = np.zeros((128, SRC_PAD), ml_dtypes.bfloat16)
    xt[:, :N] = np.asarray(x, np.float32).T.astype(ml_dtypes.bfloat16)
    wb = np.asarray(weight, np.float32).astype(ml_dtypes.bfloat16)
    bias = np.asarray(bias, np.float32).reshape(1, D)
    gamma = np.asarray(gamma, np.float32).reshape(1, D)
    beta = np.asarray(beta, np.float32).reshape(1, D)
    maps = []
    for c in range(C):
        maps.append({
            "xt": xt, "w": wb, "bias": bias, "gamma": gamma, "beta": beta,
            "st": np.ascontiguousarray(st[c]),
            "idx16": np.ascontiguousarray(idx16[c]),
        })
    return maps


_CACHE = {}


def _get_compiled(edge_index, edge_weight):
    P, T, st, idx16 = _preprocess(edge_index, edge_weight)
    key = P.tobytes()
    if key not in _CACHE:
        tiles, t0w, Tw = _schedule(P)
        _CACHE[key] = _build(P, T, tiles, t0w, Tw)
    return _CACHE[key], st, idx16


def kernel(x, edge_index, edge_weight, weight, adaptive_weight, bias,
           ln_gamma, ln_beta):
    nc, st, idx16 = _get_compiled(edge_index, edge_weight)
    in_maps = _core_maps(x, weight, bias, ln_gamma, ln_beta, st, idx16)
    res = run_bass_kernel_spmd(nc, in_maps, core_ids=list(range(C)))
    out = np.empty((N, D), np.float32)
    for c in range(C):
        out[c * NPC:(c + 1) * NPC] = res.results[c]["out"][:NPC]
    return out


# revision 37
# speedup vs baseline: 1.0206x; 1.0023x over previous
"""AGCNConv (GNN message passing) distributed Bass kernel for 8 TRN2 NeuronCores.

Reference math:
    h   = x @ W
    aew = edge_weight * sigmoid(adaptive_weight)
    deg = segment_sum(aew, row);  dis = where(deg>0, deg^-1/2, 0)
    out = segment_sum(h[col] * (dis[row]*aew*dis[col])[:,None], row)
    out = LeakyReLU(LayerNorm(out + bias))

Key identities exploited:
  * The global factor s = sigmoid(adaptive_weight) cancels in the symmetric
    normalization, so adaptive_weight is unused.
  * norm_e = dis[row]*ew*dis[col] depends only on edge data -> computed on
    the HOST in _preprocess; the device never computes degrees.

v3 design (no collectives): every core receives the FULL transposed node
matrix xT (bf16, replicated input) and redundantly computes h = x @ W for
all 40960 padded source rows, writing bf16 h to its local DRAM in two
20480-row windows (the int16 gather-index limit). Dest nodes are sharded
8 ways (5120 padded rows per core); edges are routed to their dest core
and grouped by (source window, dest block of 128), sorted by source row
within each group (gather locality), padded to 128-edge tiles with a
schedule shared across all cores (SPMD: one program, per-core tables).

The central hardware constraint (measured): DVE and GPSIMD arbitrate an
EXCLUSIVE shared SBUF port, and SWDGE gather-descriptor generation runs
on GPSIMD — any steady-state DVE work starves the gather DMA (kernel went
866us -> 560us when DVE left the loop). Hence:
  * S matrices (S[e,d] = norm_e * onehot(dest_off_e)) are built on the
    HOST and streamed via HWDGE as bf16 tiles — no DVE S-build.
  * All epilogue DVE ops are data-gated (via bias_gated) on the LAST
    gather so they cannot run mid-pass.

Device pipeline per core (emitted p1A, edgeA, p1B, edgeB so PE's in-order
queue never parks edge matmuls behind phase-1 work):
  1. h window w: xT chunks DMA'd in, 4 matmuls per [128,512] PSUM bank
     (lhsT = xT chunk, rhs = W, both bf16), one ACT copy -> bf16 staging,
     one DMA per 512 rows into h[w] DRAM.
  2. Edge pass window w: dma_gather h[col] rows (4 SWDGE queues rotating,
     32-tile chunks, 8 buffers in flight; ~2.9-3.4 ns/row isolated, ~4
     ns/row with concurrent bulk traffic), stream S tiles via HWDGE,
     PSUM-accumulate out_blk += S^T @ G per dest block (81 ns/matmul).
     Window-0 partials park in acc_sb; window-1 chains seed from them via
     an ACT SBUF->PSUM preload (start=False accumulation).
  3. Deferred epilogue per block: +bias, LayerNorm (ACT accum_out for
     var), gamma/beta, LeakyReLU via scalar_tensor_tensor max(x, 0.2x),
     DMA out.
"""

import sys

if "/opt/trn_rl_repo" not in sys.path:
    sys.path.insert(0, "/opt/trn_rl_repo")

import numpy as np
import ml_dtypes

from concourse import bacc, tile, mybir
from concourse.bass_utils import run_bass_kernel_spmd

# ---- problem constants (hardcoded per the harness contract) ----
N = 40000
E = 640000
D = 128
C = 8                # cores
NPC = 5000           # dest nodes per core
NB = 40              # dest blocks of 128 per core
NPCP = NB * 128      # 5120 padded dest rows per core
SRC_PAD = C * NPCP   # 40960 padded source rows
# gather windows (int16 index limit caps a window at 32768 rows); symmetric
# 20480/20480 measured faster than a small-A asymmetric split
WIN_A = 20480
WSZ = [WIN_A, SRC_PAD - WIN_A]
W0 = [0, WIN_A]
LN_EPS = 1e-5
LEAKY_ALPHA = 0.2

# ---- tunables ----
SKIP = frozenset()   # ablation flags: p1, g, sb, mm, epi
CH = 32              # gather chunk size in 128-edge tiles (24/12 and 64/4
NQ = 4               # variants measured slower; 32/8 is the sweet spot)
GBUFS = 10           # gather buffers in flight (2.5 per queue)
XCH = 16             # node tiles per xT chunk DMA (2048 nodes)
PST = 4              # node tiles per P1 PSUM group ([128,512] bank)

_f32 = mybir.dt.float32
_bf16 = mybir.dt.bfloat16
_i16 = mybir.dt.int16


def _preprocess(edge_index, edge_weight):
    """Host: symmetric normalization, edge routing/grouping, shared padded
    schedule, per-core tile-layout tables (ewt/doft/idx16)."""
    row = np.asarray(edge_index[0], dtype=np.int64)
    col = np.asarray(edge_index[1], dtype=np.int64)
    ew = np.asarray(edge_weight, dtype=np.float32)
    deg = np.bincount(row, weights=ew, minlength=N)
    dis = np.where(deg > 0, deg ** -0.5, 0.0)
    ew = (dis[row] * ew * dis[col]).astype(np.float32)

    core = row // NPC
    lid = row - core * NPC
    blk = lid >> 7
    off = (lid & 127).astype(np.float32)
    win = (col >= WIN_A).astype(np.int64)     # source window 0|1
    crel = (col - win * WIN_A).astype(np.int16)  # 0..32767

    # group key per edge: (core, window, block)
    key = (core * 2 + win) * NB + blk
    counts = np.bincount(key, minlength=C * 2 * NB).reshape(C, 2, NB)
    # shared schedule: tiles per (window, block) = max over cores, rounded up
    P = 128 * np.ceil(counts.max(axis=0) / 128.0).astype(np.int64)  # [2, NB]
    for b in range(NB):
        if P[0, b] + P[1, b] == 0:
            P[1, b] = 128  # at least one (all-dummy) tile per block

    Ppad = int(P.sum())
    T = Ppad // 128
    gstart = np.concatenate([[0], np.cumsum(P.reshape(-1))])[:-1].reshape(2, NB)

    colr_a = np.zeros((C, Ppad), np.int16)
    ew_a = np.zeros((C, Ppad), np.float32)
    dof_a = np.zeros((C, Ppad), np.float32)

    # sort by (group, source row): order within a group is free, ascending
    # source addresses give the gather slightly better DRAM locality
    order = np.lexsort((crel, key))
    key_s = key[order]
    grp_sizes = counts.reshape(-1)
    grp_off = np.concatenate([[0], np.cumsum(grp_sizes)])[:-1]
    within = np.arange(len(key_s)) - grp_off[key_s]
    c_s = key_s // (2 * NB)
    wb_s = key_s % (2 * NB)
    dest = gstart.reshape(-1)[wb_s] + within
    colr_a[c_s, dest] = crel[order]
    ew_a[c_s, dest] = ew[order]
    dof_a[c_s, dest] = off[order]

    # S tiles precomputed on HOST (static edge data): S[e, d] = norm_e if
    # dest_off_e == d else 0.  Layout [C, 128 edge, T*128] bf16 so tile t is
    # the column slice [:, 128t:128(t+1)] — streamed to SBUF and fed to the
    # PE as lhsT directly.  This keeps DVE silent during the edge pass: DVE
    # work would lock GPSIMD out of the shared SBUF port it needs to write
    # SWDGE gather descriptors, starving the gather DMA.
    st = np.zeros((C, 128, T * 128), ml_dtypes.bfloat16)
    cc = c_s
    tt = dest // 128
    ee = (dest % 128).astype(np.int64)
    dd = dof_a[c_s, dest].astype(np.int64)
    st[cc, ee, tt * 128 + dd] = ew[order].astype(ml_dtypes.bfloat16)
    # dma_gather index layout: idx i of a tile at [i%16, i//16], replicated x8
    A = colr_a.reshape(C, T, 8, 16).transpose(0, 1, 3, 2)                      # [C,T,16,8]
    idx16 = np.tile(A, (1, 1, 8, 1)).transpose(0, 2, 1, 3).reshape(C, 128, 8 * T)
    return P, T, st, np.ascontiguousarray(idx16)


def _schedule(P):
    """Static tile schedule shared by all cores."""
    P = np.asarray(P)
    tiles = []  # global tile idx -> (w, b)
    for w in (0, 1):
        for b in range(NB):
            for _ in range(int(P[w, b]) // 128):
                tiles.append((w, b))
    T = len(tiles)
    t0w = [0, int(P[0].sum()) // 128]
    Tw = [t0w[1], T - t0w[1]]
    return tiles, t0w, Tw


def _build(P, T, tiles, t0w, Tw, sim_single_core=False, reps=1):
    del sim_single_core  # v2 has no collectives; kept for test.py compat
    nc = bacc.Bacc("TRN2", target_bir_lowering=False, debug=False,
                   enable_asserts=True, num_devices=1,
                   num_swdge_queues=NQ, dynamic_dma_scratch_size=65536)

    xt_in = nc.dram_tensor("xt", [128, SRC_PAD], _bf16, kind="ExternalInput").ap()
    w_in = nc.dram_tensor("w", [D, D], _bf16, kind="ExternalInput").ap()
    bias_in = nc.dram_tensor("bias", [1, D], _f32, kind="ExternalInput").ap()
    gamma_in = nc.dram_tensor("gamma", [1, D], _f32, kind="ExternalInput").ap()
    beta_in = nc.dram_tensor("beta", [1, D], _f32, kind="ExternalInput").ap()
    st_in = nc.dram_tensor("st", [128, 128 * T], _bf16, kind="ExternalInput").ap()
    idx_in = nc.dram_tensor("idx16", [128, 8 * T], _i16, kind="ExternalInput").ap()
    out_d = nc.dram_tensor("out", [NPCP, D], _f32, kind="ExternalOutput").ap()

    eq = mybir.AluOpType.is_equal
    mul = mybir.AluOpType.mult
    add = mybir.AluOpType.add
    AF = mybir.ActivationFunctionType

    # per-block window runs: (first_tile, last_tile) or None
    runs = [[None, None] for _ in range(NB)]
    for t, (w, b) in enumerate(tiles):
        if runs[b][w] is None:
            runs[b][w] = [t, t]
        else:
            runs[b][w][1] = t

    with tile.TileContext(nc) as tc:
        with (
            tc.tile_pool(name="const", bufs=1) as cp,
            tc.tile_pool(name="resident", bufs=1) as rp,
            tc.tile_pool(name="dram", bufs=1, space="DRAM") as dp,
        ):
            ones_row = cp.tile([1, 128], _f32)
            nc.vector.memset(ones_row[:], 1.0)
            eps_col = cp.tile([128, 1], _f32)
            nc.vector.memset(eps_col[:], float(LN_EPS))
            w_sb = cp.tile([128, 128], _bf16)
            nc.sync.dma_start(w_sb[:], w_in)

            # broadcast bias/gamma/beta rows to all 128 partitions via matmul
            bias_bc = cp.tile([128, 128], _f32)
            gamma_bc = cp.tile([128, 128], _f32)
            beta_bc = cp.tile([128, 128], _f32)
            with tc.tile_pool(name="bc", bufs=1) as bcp, \
                 tc.tile_pool(name="bcps", bufs=1, space="PSUM") as bcps:
                for src_ap, dst in ((bias_in, bias_bc), (gamma_in, gamma_bc),
                                    (beta_in, beta_bc)):
                    r = bcp.tile([1, 128], _f32, tag="bcrow")
                    nc.sync.dma_start(r[:], src_ap)
                    ps = bcps.tile([128, 128], _f32, tag="bcps")
                    nc.tensor.matmul(ps[:], lhsT=ones_row[:], rhs=r[:],
                                     start=True, stop=True)
                    nc.scalar.copy(dst[:], ps[:])

            idx_sb = rp.tile([128, 8 * T], _i16)
            nc.sync.dma_start(idx_sb[:], idx_in)

            acc_sb = rp.tile([128, NB, 128], _f32)  # window-0 partial sums

            h_a = dp.tile([WSZ[0], D], _bf16)
            h_b = dp.tile([WSZ[1], D], _bf16)
            h_d = [h_a, h_b]

            dmaeng = [nc.sync, nc.scalar]  # HWDGE streams for plain DMAs

            def _phases():
                # ---------------- Phase 1: h[w] = bf16(x @ W) ----------------
                def p1_window(w):
                    if "p1" in SKIP:
                        # ablation: fill h with arbitrary bytes in one DMA so
                        # gather reads aren't reads-before-any-write
                        hv = h_d[w][:].rearrange("(t p) d -> p t d", p=128)
                        src = xt_in[:, W0[w]:W0[w] + WSZ[w]].rearrange(
                            "p (t d) -> p t d", d=128)
                        nc.sync.dma_start(hv, src)
                        return
                    nchunks = WSZ[w] // (XCH * 128)
                    for ci in range(nchunks):
                        base = W0[w] + ci * XCH * 128
                        xc = p1x.tile([128, XCH * 128], _bf16, tag="xc")
                        dmaeng[ci % 2].dma_start(
                            xc[:], xt_in[:, base:base + XCH * 128])
                        for g in range(XCH // PST):
                            ps = p1ps.tile([128, PST * 128], _f32, tag="hps")
                            for j in range(PST):
                                k = (g * PST + j) * 128
                                nc.tensor.matmul(
                                    ps[:, j * 128:(j + 1) * 128],
                                    lhsT=xc[:, k:k + 128], rhs=w_sb[:],
                                    start=True, stop=True)
                            hst = p1h.tile([128, PST, 128], _bf16, tag="hst")
                            nc.scalar.copy(hst[:], ps[:])
                            r0 = ci * XCH * 128 + g * PST * 128
                            dst = h_d[w][r0:r0 + PST * 128, :].rearrange(
                                "(t p) d -> p t d", p=128)
                            dmaeng[(ci + g) % 2].dma_start(dst, hst[:])

                # ---------------- Edge pass: gather + scatter matmuls --------
                def epilogue(b, bias_src):
                    # gated on bias_src (copied only after the final gather):
                    # epilogue DVE ops would otherwise run mid-pass and lock
                    # GPSIMD out of SWDGE gather-descriptor generation
                    if "epi" in SKIP:
                        return
                    t4 = ep.tile([128, 128], _f32, tag="e_t4")
                    nc.vector.tensor_tensor(t4[:], acc_sb[:, b, :],
                                            bias_src[:], op=add)
                    nsum = ep.tile([128, 1], _f32, tag="e_ns")
                    nc.vector.tensor_reduce(nsum[:], t4[:],
                                            axis=mybir.AxisListType.X,
                                            op=add, negate=True)
                    nmean = ep.tile([128, 1], _f32, tag="e_nm")
                    nc.scalar.mul(nmean[:], nsum[:], 1.0 / 128.0)
                    t5 = ep.tile([128, 128], _f32, tag="e_t5")
                    nc.scalar.activation(t5[:], t4[:], AF.Identity,
                                         bias=nmean[:], scale=1.0)
                    sq = ep.tile([128, 128], _f32, tag="e_sq")
                    vsum = ep.tile([128, 1], _f32, tag="e_vs")
                    nc.scalar.activation(sq[:], t5[:], AF.Square,
                                         accum_out=vsum[:])
                    sd = ep.tile([128, 1], _f32, tag="e_sd")
                    nc.scalar.activation(sd[:], vsum[:], AF.Sqrt,
                                         scale=1.0 / 128.0, bias=eps_col[:])
                    rstd = ep.tile([128, 1], _f32, tag="e_rs")
                    nc.vector.reciprocal(rstd[:], sd[:])
                    t6 = ep.tile([128, 128], _f32, tag="e_t6")
                    nc.vector.scalar_tensor_tensor(t6[:], t5[:], rstd[:],
                                                   gamma_bc[:], op0=mul, op1=mul)
                    nc.vector.tensor_tensor(t6[:], t6[:], beta_bc[:], op=add)
                    osb = ep.tile([128, 128], _f32, tag="e_o")
                    nc.vector.scalar_tensor_tensor(osb[:], t6[:],
                                                   float(LEAKY_ALPHA), t6[:],
                                                   op0=mul,
                                                   op1=mybir.AluOpType.max)
                    nc.sync.dma_start(out_d[b * 128:(b + 1) * 128, :], osb[:])

                gcall = [0]

                def edge_window(w):
                    win_ap = h_d[w][:]
                    nt_left = Tw[w]
                    t0 = t0w[w]
                    cur_ps = None
                    while nt_left > 0:
                        nt = min(CH, nt_left)
                        gbuf = gbp.tile([128, CH, 128], _bf16, tag="g")
                        if "gs" in SKIP:
                            # ablation: same bytes, sequential HWDGE DMA
                            src = h_d[w][:nt * 128, :].rearrange(
                                "(t p) d -> p t d", p=128)
                            dmaeng[gcall[0] % 2].dma_start(gbuf[:, :nt, :], src)
                            gcall[0] += 1
                        elif "g" in SKIP:
                            nc.vector.memset(gbuf[:], 0.0)
                        else:
                            nc.gpsimd.dma_gather(
                                out_ap=gbuf[:, :nt, :], in_ap=win_ap,
                                idxs_ap=idx_sb[:, 8 * t0:8 * (t0 + nt)],
                                num_idxs=128 * nt, num_idxs_reg=128 * nt,
                                elem_size=128, single_packet=False,
                                queue_num=gcall[0] % NQ)
                            gcall[0] += 1
                        if "sb" not in SKIP:  # "sb" skip implies "mm" skip
                            st_sb = stp.tile([128, CH * 128], _bf16, tag="st")
                            dmaeng[gcall[0] % 2].dma_start(
                                st_sb[:, :nt * 128],
                                st_in[:, 128 * t0:128 * (t0 + nt)])
                        for s_i in range(nt):
                            t = t0 + s_i
                            b = tiles[t][1]
                            first = runs[b][w][0] == t
                            last = runs[b][w][1] == t
                            preload = w == 1 and runs[b][0] is not None
                            if first:
                                cur_ps = pbps.tile([128, 128], _f32, tag="blk")
                                if preload and "mm" not in SKIP:
                                    # seed the accumulation with the window-0
                                    # partial so no separate add is needed
                                    nc.scalar.copy(cur_ps[:], acc_sb[:, b, :])
                                if "mm" in SKIP:
                                    nc.vector.memset(cur_ps[:], 0.0)
                            if "mm" not in SKIP:
                                nc.tensor.matmul(
                                    cur_ps[:],
                                    lhsT=st_sb[:, 128 * s_i:128 * (s_i + 1)],
                                    rhs=gbuf[:, s_i, :],
                                    start=first and not preload, stop=last)
                            if last:
                                nc.scalar.copy(acc_sb[:, b, :], cur_ps[:])
                                cur_ps = None
                        t0 += nt
                        nt_left -= nt

                with tc.tile_pool(name="p1x", bufs=2) as p1x, \
                     tc.tile_pool(name="p1h", bufs=3) as p1h, \
                     tc.tile_pool(name="p1ps", bufs=3, space="PSUM") as p1ps, \
                     tc.tile_pool(name="stp", bufs=3) as stp, \
                     tc.tile_pool(name="gb", bufs=GBUFS) as gbp, \
                     tc.tile_pool(name="pbps", bufs=4, space="PSUM") as pbps, \
                     tc.tile_pool(name="ep", bufs=2) as ep:
                    # p1(0), edge(0), p1(1), edge(1): keeps PE's in-order
                    # queue from parking edge-A matmuls behind P1-B, which
                    # would stall gather-A once the gather buffers fill
                    p1_window(0)
                    edge_window(0)
                    p1_window(1)
                    edge_window(1)
                    # bias_gated is written after the last edge-pass psum
                    # copy in ACT program order; every epilogue reads it, so
                    # no epilogue DVE op can start before the gathers end
                    bias_gated = ep.tile([128, 128], _f32, tag="e_bg")
                    nc.scalar.copy(bias_gated[:], bias_bc[:])
                    for b in range(NB):
                        epilogue(b, bias_gated)

            if reps == 1:
                _phases()
            else:
                with tc.For_i(0, reps, 1):
                    _phases()

    nc.compile()
    return nc


def _core_maps(x, weight, bias, gamma, beta, st, idx16):
    """Per-core input maps (shared by kernel() and test.py)."""
    xt = np.zeros((128, SRC_PAD), ml_dtypes.bfloat16)
    xt[:, :N] = np.asarray(x, np.float32).T.astype(ml_dtypes.bfloat16)
    wb = np.asarray(weight, np.float32).astype(ml_dtypes.bfloat16)
    bias = np.asarray(bias, np.float32).reshape(1, D)
    gamma = np.asarray(gamma, np.float32).reshape(1, D)
    beta = np.asarray(beta, np.float32).reshape(1, D)
    maps = []
    for c in range(C):
        maps.append({
            "xt": xt, "w": wb, "bias": bias, "gamma": gamma, "beta": beta,
            "st": np.ascontiguousarray(st[c]),
            "idx16": np.ascontiguousarray(idx16[c]),
        })
    return maps


_CACHE = {}


def _get_compiled(edge_index, edge_weight):
    P, T, st, idx16 = _preprocess(edge_index, edge_weight)
    key = P.tobytes()
    if key not in _CACHE:
        tiles, t0w, Tw = _schedule(P)
        _CACHE[key] = _build(P, T, tiles, t0w, Tw)
    return _CACHE[key], st, idx16


def kernel(x, edge_index, edge_weight, weight, adaptive_weight, bias,
           ln_gamma, ln_beta):
    nc, st, idx16 = _get_compiled(edge_index, edge_weight)
    in_maps = _core_maps(x, weight, bias, ln_gamma, ln_beta, st, idx16)
    res = run_bass_kernel_spmd(nc, in_maps, core_ids=list(range(C)))
    out = np.empty((N, D), np.float32)
    for c in range(C):
        out[c * NPC:(c + 1) * NPC] = res.results[c]["out"][:NPC]
    return out


# revision 40
# speedup vs baseline: 1.1167x; 1.0941x over previous
"""AGCNConv (GNN message passing) distributed Bass kernel for 8 TRN2 NeuronCores.

Reference math:
    h   = x @ W
    aew = edge_weight * sigmoid(adaptive_weight)
    deg = segment_sum(aew, row);  dis = where(deg>0, deg^-1/2, 0)
    out = segment_sum(h[col] * (dis[row]*aew*dis[col])[:,None], row)
    out = LeakyReLU(LayerNorm(out + bias))

Key identities exploited:
  * The global factor s = sigmoid(adaptive_weight) cancels in the symmetric
    normalization, so adaptive_weight is unused.
  * norm_e = dis[row]*ew*dis[col] depends only on edge data -> computed on
    the HOST in _preprocess; the device never computes degrees.

v3 design (no collectives): every core receives the FULL transposed node
matrix xT (bf16, replicated input) and redundantly computes h = x @ W for
all 40960 padded source rows, writing bf16 h to its local DRAM in two
20480-row windows (the int16 gather-index limit). Dest nodes are sharded
8 ways (5120 padded rows per core); edges are routed to their dest core
and grouped by (source window, dest block of 128), sorted by source row
within each group (gather locality), padded to 128-edge tiles with a
schedule shared across all cores (SPMD: one program, per-core tables).

The central hardware constraint (measured): DVE and GPSIMD arbitrate an
EXCLUSIVE shared SBUF port, and SWDGE gather-descriptor generation runs
on GPSIMD — any steady-state DVE work starves the gather DMA (kernel went
866us -> 560us when DVE left the loop). Hence:
  * S matrices (S[e,d] = norm_e * onehot(dest_off_e)) are built on the
    HOST and streamed via HWDGE as bf16 tiles — no DVE S-build.
  * All epilogue DVE ops are data-gated (via bias_gated) on the LAST
    gather so they cannot run mid-pass.

Device pipeline per core (emitted p1A, edgeA, p1B, edgeB so PE's in-order
queue never parks edge matmuls behind phase-1 work):
  1. h window w: xT chunks DMA'd in, 4 matmuls per [128,512] PSUM bank
     (lhsT = xT chunk, rhs = W, both bf16), one ACT copy -> bf16 staging,
     one DMA per 512 rows into h[w] DRAM.
  2. Edge pass window w: dma_gather h[col] rows (4 SWDGE queues rotating,
     32-tile chunks, 8 buffers in flight; ~2.9-3.4 ns/row isolated, ~4
     ns/row with concurrent bulk traffic), stream S tiles via HWDGE,
     PSUM-accumulate out_blk += S^T @ G per dest block (81 ns/matmul).
     Window-0 partials park in acc_sb; window-1 chains seed from them via
     an ACT SBUF->PSUM preload (start=False accumulation).
  3. Deferred epilogue per block: +bias, LayerNorm (ACT accum_out for
     var), gamma/beta, LeakyReLU via scalar_tensor_tensor max(x, 0.2x),
     DMA out.
"""

import sys

if "/opt/trn_rl_repo" not in sys.path:
    sys.path.insert(0, "/opt/trn_rl_repo")

import numpy as np
import ml_dtypes

from concourse import bacc, tile, mybir
from concourse.bass_utils import run_bass_kernel_spmd

# ---- problem constants (hardcoded per the harness contract) ----
N = 40000
E = 640000
D = 128
C = 8                # cores
NPC = 5000           # dest nodes per core
NB = 40              # dest blocks of 128 per core
NPCP = NB * 128      # 5120 padded dest rows per core
SRC_PAD = C * NPCP   # 40960 padded source rows
# gather windows (int16 index limit caps a window at 32768 rows); symmetric
# 20480/20480 measured faster than a small-A asymmetric split
WIN_A = 20480
WSZ = [WIN_A, SRC_PAD - WIN_A]
W0 = [0, WIN_A]
LN_EPS = 1e-5
LEAKY_ALPHA = 0.2

# ---- tunables ----
SKIP = frozenset()   # ablation flags: p1, g, sb, mm, epi
CH = 32              # gather chunk size in 128-edge tiles (24/12 and 64/4
NQ = 4               # variants measured slower; 32/8 is the sweet spot)
GBUFS = 8            # gather buffers in flight (2/queue; 10 and 12 measured
                     # identical-to-worse — the gather is not slot-starved)
XCH = 16             # node tiles per xT chunk DMA (2048 nodes)
PST = 4              # node tiles per P1 PSUM group ([128,512] bank)

_f32 = mybir.dt.float32
_bf16 = mybir.dt.bfloat16
_i16 = mybir.dt.int16


def _preprocess(edge_index, edge_weight):
    """Host: symmetric normalization, edge routing/grouping, shared padded
    schedule, per-core tile-layout tables (ewt/doft/idx16)."""
    row = np.asarray(edge_index[0], dtype=np.int64)
    col = np.asarray(edge_index[1], dtype=np.int64)
    ew = np.asarray(edge_weight, dtype=np.float32)
    deg = np.bincount(row, weights=ew, minlength=N)
    dis = np.where(deg > 0, deg ** -0.5, 0.0)
    ew = (dis[row] * ew * dis[col]).astype(np.float32)

    core = row // NPC
    lid = row - core * NPC
    win = (col >= WIN_A).astype(np.int64)     # source window 0|1
    crel = (col - win * WIN_A).astype(np.int16)  # 0..32767

    # Dest rows are assigned to blocks by BALANCED bin-packing (not lid>>7):
    # block membership within a core's output is free as long as the host
    # un-permutes afterwards, and balancing both windows' per-block edge
    # counts under 1024 gives every (core,window,block) group 8 tiles
    # instead of a variance-padded 9 -- ~8% fewer gathered rows.
    blk_of = np.zeros((C, NPC), np.int64)
    pos_of = np.zeros((C, NPC), np.int64)
    for c in range(C):
        m = core == c
        lm, wm = lid[m], win[m]
        na = np.bincount(lm[wm == 0], minlength=NPC).astype(np.float64)
        nb = np.bincount(lm[wm == 1], minlength=NPC).astype(np.float64)
        order_r = np.argsort(-(na + nb), kind="stable")
        sums = np.zeros((NB, 2))
        cnt = np.zeros(NB, np.int64)
        for r in order_r:
            load = np.maximum(sums[:, 0] + na[r], sums[:, 1] + nb[r])
            load[cnt >= 128] = np.inf
            b = int(np.argmin(load))
            blk_of[c, r] = b
            pos_of[c, r] = cnt[b]
            cnt[b] += 1
            sums[b, 0] += na[r]
            sums[b, 1] += nb[r]
    blk = blk_of[core, lid]
    off = pos_of[core, lid].astype(np.float32)
    # outmap[c, 128*b + pos] = original lid (or -1 for pad slots)
    outmap = np.full((C, NPCP), -1, np.int64)
    for c in range(C):
        outmap[c, 128 * blk_of[c] + pos_of[c]] = np.arange(NPC)
    global _OUTMAP
    _OUTMAP = outmap

    # group key per edge: (core, window, block)
    key = (core * 2 + win) * NB + blk
    counts = np.bincount(key, minlength=C * 2 * NB).reshape(C, 2, NB)
    # shared schedule: tiles per (window, block) = max over cores, rounded up
    P = 128 * np.ceil(counts.max(axis=0) / 128.0).astype(np.int64)  # [2, NB]
    for b in range(NB):
        if P[0, b] + P[1, b] == 0:
            P[1, b] = 128  # at least one (all-dummy) tile per block

    Ppad = int(P.sum())
    T = Ppad // 128
    gstart = np.concatenate([[0], np.cumsum(P.reshape(-1))])[:-1].reshape(2, NB)

    colr_a = np.zeros((C, Ppad), np.int16)
    ew_a = np.zeros((C, Ppad), np.float32)
    dof_a = np.zeros((C, Ppad), np.float32)

    # sort by (group, source row): order within a group is free, ascending
    # source addresses give the gather slightly better DRAM locality
    order = np.lexsort((crel, key))
    key_s = key[order]
    grp_sizes = counts.reshape(-1)
    grp_off = np.concatenate([[0], np.cumsum(grp_sizes)])[:-1]
    within = np.arange(len(key_s)) - grp_off[key_s]
    c_s = key_s // (2 * NB)
    wb_s = key_s % (2 * NB)
    dest = gstart.reshape(-1)[wb_s] + within
    colr_a[c_s, dest] = crel[order]
    ew_a[c_s, dest] = ew[order]
    dof_a[c_s, dest] = off[order]

    # S tiles precomputed on HOST (static edge data): S[e, d] = norm_e if
    # dest_off_e == d else 0.  Layout [C, 128 edge, T*128] bf16 so tile t is
    # the column slice [:, 128t:128(t+1)] — streamed to SBUF and fed to the
    # PE as lhsT directly.  This keeps DVE silent during the edge pass: DVE
    # work would lock GPSIMD out of the shared SBUF port it needs to write
    # SWDGE gather descriptors, starving the gather DMA.
    st = np.zeros((C, 128, T * 128), ml_dtypes.bfloat16)
    cc = c_s
    tt = dest // 128
    ee = (dest % 128).astype(np.int64)
    dd = dof_a[c_s, dest].astype(np.int64)
    st[cc, ee, tt * 128 + dd] = ew[order].astype(ml_dtypes.bfloat16)
    # dma_gather index layout: idx i of a tile at [i%16, i//16], replicated x8
    A = colr_a.reshape(C, T, 8, 16).transpose(0, 1, 3, 2)                      # [C,T,16,8]
    idx16 = np.tile(A, (1, 1, 8, 1)).transpose(0, 2, 1, 3).reshape(C, 128, 8 * T)
    return P, T, st, np.ascontiguousarray(idx16)


def _schedule(P):
    """Static tile schedule shared by all cores."""
    P = np.asarray(P)
    tiles = []  # global tile idx -> (w, b)
    for w in (0, 1):
        for b in range(NB):
            for _ in range(int(P[w, b]) // 128):
                tiles.append((w, b))
    T = len(tiles)
    t0w = [0, int(P[0].sum()) // 128]
    Tw = [t0w[1], T - t0w[1]]
    return tiles, t0w, Tw


def _build(P, T, tiles, t0w, Tw, sim_single_core=False, reps=1):
    del sim_single_core  # v2 has no collectives; kept for test.py compat
    nc = bacc.Bacc("TRN2", target_bir_lowering=False, debug=False,
                   enable_asserts=True, num_devices=1,
                   num_swdge_queues=NQ, dynamic_dma_scratch_size=65536)

    xt_in = nc.dram_tensor("xt", [128, SRC_PAD], _bf16, kind="ExternalInput").ap()
    w_in = nc.dram_tensor("w", [D, D], _bf16, kind="ExternalInput").ap()
    bias_in = nc.dram_tensor("bias", [1, D], _f32, kind="ExternalInput").ap()
    gamma_in = nc.dram_tensor("gamma", [1, D], _f32, kind="ExternalInput").ap()
    beta_in = nc.dram_tensor("beta", [1, D], _f32, kind="ExternalInput").ap()
    st_in = nc.dram_tensor("st", [128, 128 * T], _bf16, kind="ExternalInput").ap()
    idx_in = nc.dram_tensor("idx16", [128, 8 * T], _i16, kind="ExternalInput").ap()
    out_d = nc.dram_tensor("out", [NPCP, D], _f32, kind="ExternalOutput").ap()

    eq = mybir.AluOpType.is_equal
    mul = mybir.AluOpType.mult
    add = mybir.AluOpType.add
    AF = mybir.ActivationFunctionType

    # per-block window runs: (first_tile, last_tile) or None
    runs = [[None, None] for _ in range(NB)]
    for t, (w, b) in enumerate(tiles):
        if runs[b][w] is None:
            runs[b][w] = [t, t]
        else:
            runs[b][w][1] = t

    with tile.TileContext(nc) as tc:
        with (
            tc.tile_pool(name="const", bufs=1) as cp,
            tc.tile_pool(name="resident", bufs=1) as rp,
            tc.tile_pool(name="dram", bufs=1, space="DRAM") as dp,
        ):
            ones_row = cp.tile([1, 128], _f32)
            nc.vector.memset(ones_row[:], 1.0)
            eps_col = cp.tile([128, 1], _f32)
            nc.vector.memset(eps_col[:], float(LN_EPS))
            w_sb = cp.tile([128, 128], _bf16)
            nc.sync.dma_start(w_sb[:], w_in)

            # broadcast bias/gamma/beta rows to all 128 partitions via matmul
            bias_bc = cp.tile([128, 128], _f32)
            gamma_bc = cp.tile([128, 128], _f32)
            beta_bc = cp.tile([128, 128], _f32)
            with tc.tile_pool(name="bc", bufs=1) as bcp, \
                 tc.tile_pool(name="bcps", bufs=1, space="PSUM") as bcps:
                for src_ap, dst in ((bias_in, bias_bc), (gamma_in, gamma_bc),
                                    (beta_in, beta_bc)):
                    r = bcp.tile([1, 128], _f32, tag="bcrow")
                    nc.sync.dma_start(r[:], src_ap)
                    ps = bcps.tile([128, 128], _f32, tag="bcps")
                    nc.tensor.matmul(ps[:], lhsT=ones_row[:], rhs=r[:],
                                     start=True, stop=True)
                    nc.scalar.copy(dst[:], ps[:])

            idx_sb = rp.tile([128, 8 * T], _i16)
            nc.sync.dma_start(idx_sb[:], idx_in)

            acc_sb = rp.tile([128, NB, 128], _f32)  # window-0 partial sums

            h_a = dp.tile([WSZ[0], D], _bf16)
            h_b = dp.tile([WSZ[1], D], _bf16)
            h_d = [h_a, h_b]

            dmaeng = [nc.sync, nc.scalar]  # HWDGE streams for plain DMAs

            def _phases():
                # ---------------- Phase 1: h[w] = bf16(x @ W) ----------------
                def p1_window(w):
                    if "p1" in SKIP:
                        # ablation: fill h with arbitrary bytes in one DMA so
                        # gather reads aren't reads-before-any-write
                        hv = h_d[w][:].rearrange("(t p) d -> p t d", p=128)
                        src = xt_in[:, W0[w]:W0[w] + WSZ[w]].rearrange(
                            "p (t d) -> p t d", d=128)
                        nc.sync.dma_start(hv, src)
                        return
                    nchunks = WSZ[w] // (XCH * 128)
                    for ci in range(nchunks):
                        base = W0[w] + ci * XCH * 128
                        xc = p1x.tile([128, XCH * 128], _bf16, tag="xc")
                        dmaeng[ci % 2].dma_start(
                            xc[:], xt_in[:, base:base + XCH * 128])
                        for g in range(XCH // PST):
                            ps = p1ps.tile([128, PST * 128], _f32, tag="hps")
                            for j in range(PST):
                                k = (g * PST + j) * 128
                                nc.tensor.matmul(
                                    ps[:, j * 128:(j + 1) * 128],
                                    lhsT=xc[:, k:k + 128], rhs=w_sb[:],
                                    start=True, stop=True)
                            hst = p1h.tile([128, PST, 128], _bf16, tag="hst")
                            nc.scalar.copy(hst[:], ps[:])
                            r0 = ci * XCH * 128 + g * PST * 128
                            dst = h_d[w][r0:r0 + PST * 128, :].rearrange(
                                "(t p) d -> p t d", p=128)
                            dmaeng[(ci + g) % 2].dma_start(dst, hst[:])

                # ---------------- Edge pass: gather + scatter matmuls --------
                def epilogue(b, bias_src):
                    # gated on bias_src (copied only after the final gather):
                    # epilogue DVE ops would otherwise run mid-pass and lock
                    # GPSIMD out of SWDGE gather-descriptor generation
                    if "epi" in SKIP:
                        return
                    t4 = ep.tile([128, 128], _f32, tag="e_t4")
                    nc.vector.tensor_tensor(t4[:], acc_sb[:, b, :],
                                            bias_src[:], op=add)
                    nsum = ep.tile([128, 1], _f32, tag="e_ns")
                    nc.vector.tensor_reduce(nsum[:], t4[:],
                                            axis=mybir.AxisListType.X,
                                            op=add, negate=True)
                    nmean = ep.tile([128, 1], _f32, tag="e_nm")
                    nc.scalar.mul(nmean[:], nsum[:], 1.0 / 128.0)
                    t5 = ep.tile([128, 128], _f32, tag="e_t5")
                    nc.scalar.activation(t5[:], t4[:], AF.Identity,
                                         bias=nmean[:], scale=1.0)
                    sq = ep.tile([128, 128], _f32, tag="e_sq")
                    vsum = ep.tile([128, 1], _f32, tag="e_vs")
                    nc.scalar.activation(sq[:], t5[:], AF.Square,
                                         accum_out=vsum[:])
                    sd = ep.tile([128, 1], _f32, tag="e_sd")
                    nc.scalar.activation(sd[:], vsum[:], AF.Sqrt,
                                         scale=1.0 / 128.0, bias=eps_col[:])
                    rstd = ep.tile([128, 1], _f32, tag="e_rs")
                    nc.vector.reciprocal(rstd[:], sd[:])
                    t6 = ep.tile([128, 128], _f32, tag="e_t6")
                    nc.vector.scalar_tensor_tensor(t6[:], t5[:], rstd[:],
                                                   gamma_bc[:], op0=mul, op1=mul)
                    nc.vector.tensor_tensor(t6[:], t6[:], beta_bc[:], op=add)
                    osb = ep.tile([128, 128], _f32, tag="e_o")
                    nc.vector.scalar_tensor_tensor(osb[:], t6[:],
                                                   float(LEAKY_ALPHA), t6[:],
                                                   op0=mul,
                                                   op1=mybir.AluOpType.max)
                    nc.sync.dma_start(out_d[b * 128:(b + 1) * 128, :], osb[:])

                gcall = [0]

                def edge_window(w):
                    win_ap = h_d[w][:]
                    nt_left = Tw[w]
                    t0 = t0w[w]
                    cur_ps = None
                    while nt_left > 0:
                        nt = min(CH, nt_left)
                        gbuf = gbp.tile([128, CH, 128], _bf16, tag="g")
                        if "gs" in SKIP:
                            # ablation: same bytes, sequential HWDGE DMA
                            src = h_d[w][:nt * 128, :].rearrange(
                                "(t p) d -> p t d", p=128)
                            dmaeng[gcall[0] % 2].dma_start(gbuf[:, :nt, :], src)
                            gcall[0] += 1
                        elif "g" in SKIP:
                            nc.vector.memset(gbuf[:], 0.0)
                        else:
                            nc.gpsimd.dma_gather(
                                out_ap=gbuf[:, :nt, :], in_ap=win_ap,
                                idxs_ap=idx_sb[:, 8 * t0:8 * (t0 + nt)],
                                num_idxs=128 * nt, num_idxs_reg=128 * nt,
                                elem_size=128, single_packet=False,
                                queue_num=gcall[0] % NQ)
                            gcall[0] += 1
                        if "sb" not in SKIP:  # "sb" skip implies "mm" skip
                            st_sb = stp.tile([128, CH * 128], _bf16, tag="st")
                            dmaeng[gcall[0] % 2].dma_start(
                                st_sb[:, :nt * 128],
                                st_in[:, 128 * t0:128 * (t0 + nt)])
                        for s_i in range(nt):
                            t = t0 + s_i
                            b = tiles[t][1]
                            first = runs[b][w][0] == t
                            last = runs[b][w][1] == t
                            preload = w == 1 and runs[b][0] is not None
                            if first:
                                cur_ps = pbps.tile([128, 128], _f32, tag="blk")
                                if preload and "mm" not in SKIP:
                                    # seed the accumulation with the window-0
                                    # partial so no separate add is needed
                                    nc.scalar.copy(cur_ps[:], acc_sb[:, b, :])
                                if "mm" in SKIP:
                                    nc.vector.memset(cur_ps[:], 0.0)
                            if "mm" not in SKIP:
                                nc.tensor.matmul(
                                    cur_ps[:],
                                    lhsT=st_sb[:, 128 * s_i:128 * (s_i + 1)],
                                    rhs=gbuf[:, s_i, :],
                                    start=first and not preload, stop=last)
                            if last:
                                nc.scalar.copy(acc_sb[:, b, :], cur_ps[:])
                                cur_ps = None
                        t0 += nt
                        nt_left -= nt

                with tc.tile_pool(name="p1x", bufs=2) as p1x, \
                     tc.tile_pool(name="p1h", bufs=3) as p1h, \
                     tc.tile_pool(name="p1ps", bufs=3, space="PSUM") as p1ps, \
                     tc.tile_pool(name="stp", bufs=3) as stp, \
                     tc.tile_pool(name="gb", bufs=GBUFS) as gbp, \
                     tc.tile_pool(name="pbps", bufs=4, space="PSUM") as pbps, \
                     tc.tile_pool(name="ep", bufs=2) as ep:
                    # p1(0), edge(0), p1(1), edge(1): keeps PE's in-order
                    # queue from parking edge-A matmuls behind P1-B, which
                    # would stall gather-A once the gather buffers fill
                    p1_window(0)
                    edge_window(0)
                    p1_window(1)
                    edge_window(1)
                    # bias_gated is written after the last edge-pass psum
                    # copy in ACT program order; every epilogue reads it, so
                    # no epilogue DVE op can start before the gathers end
                    bias_gated = ep.tile([128, 128], _f32, tag="e_bg")
                    nc.scalar.copy(bias_gated[:], bias_bc[:])
                    for b in range(NB):
                        epilogue(b, bias_gated)

            if reps == 1:
                _phases()
            else:
                with tc.For_i(0, reps, 1):
                    _phases()

    nc.compile()
    return nc


def _core_maps(x, weight, bias, gamma, beta, st, idx16):
    """Per-core input maps (shared by kernel() and test.py)."""
    xt = np.zeros((128, SRC_PAD), ml_dtypes.bfloat16)
    xt[:, :N] = np.asarray(x, np.float32).T.astype(ml_dtypes.bfloat16)
    wb = np.asarray(weight, np.float32).astype(ml_dtypes.bfloat16)
    bias = np.asarray(bias, np.float32).reshape(1, D)
    gamma = np.asarray(gamma, np.float32).reshape(1, D)
    beta = np.asarray(beta, np.float32).reshape(1, D)
    maps = []
    for c in range(C):
        maps.append({
            "xt": xt, "w": wb, "bias": bias, "gamma": gamma, "beta": beta,
            "st": np.ascontiguousarray(st[c]),
            "idx16": np.ascontiguousarray(idx16[c]),
        })
    return maps


_CACHE = {}
_OUTMAP = None


def _get_compiled(edge_index, edge_weight):
    P, T, st, idx16 = _preprocess(edge_index, edge_weight)
    key = P.tobytes()
    if key not in _CACHE:
        tiles, t0w, Tw = _schedule(P)
        _CACHE[key] = _build(P, T, tiles, t0w, Tw)
    return _CACHE[key], st, idx16


def kernel(x, edge_index, edge_weight, weight, adaptive_weight, bias,
           ln_gamma, ln_beta):
    nc, st, idx16 = _get_compiled(edge_index, edge_weight)
    in_maps = _core_maps(x, weight, bias, ln_gamma, ln_beta, st, idx16)
    res = run_bass_kernel_spmd(nc, in_maps, core_ids=list(range(C)))
    out = np.empty((N, D), np.float32)
    for c in range(C):
        oc = np.asarray(res.results[c]["out"])
        m = _OUTMAP[c]
        valid = m >= 0
        out[c * NPC + m[valid]] = oc[valid]
    return out


# revision 41
# speedup vs baseline: 1.2192x; 1.0919x over previous
"""AGCNConv (GNN message passing) distributed Bass kernel for 8 TRN2 NeuronCores.

Reference math:
    h   = x @ W
    aew = edge_weight * sigmoid(adaptive_weight)
    deg = segment_sum(aew, row);  dis = where(deg>0, deg^-1/2, 0)
    out = segment_sum(h[col] * (dis[row]*aew*dis[col])[:,None], row)
    out = LeakyReLU(LayerNorm(out + bias))

Key identities exploited:
  * The global factor s = sigmoid(adaptive_weight) cancels in the symmetric
    normalization, so adaptive_weight is unused.
  * norm_e = dis[row]*ew*dis[col] depends only on edge data -> computed on
    the HOST in _preprocess; the device never computes degrees.

v3 design (no collectives): every core receives the FULL transposed node
matrix xT (bf16, replicated input) and redundantly computes h = x @ W for
all 40960 padded source rows, writing bf16 h to its local DRAM in two
20480-row windows (the int16 gather-index limit). Dest nodes are sharded
8 ways (5120 padded rows per core); edges are routed to their dest core
and grouped by (source window, dest block of 128), sorted by source row
within each group (gather locality), padded to 128-edge tiles with a
schedule shared across all cores (SPMD: one program, per-core tables).

The central hardware constraint (measured): DVE and GPSIMD arbitrate an
EXCLUSIVE shared SBUF port, and SWDGE gather-descriptor generation runs
on GPSIMD — any steady-state DVE work starves the gather DMA (kernel went
866us -> 560us when DVE left the loop). Hence:
  * S matrices (S[e,d] = norm_e * onehot(dest_off_e)) are built on the
    HOST and streamed via HWDGE as bf16 tiles — no DVE S-build.
  * All epilogue DVE ops are data-gated (via bias_gated) on the LAST
    gather so they cannot run mid-pass.

Device pipeline per core (emitted p1A, edgeA, p1B, edgeB so PE's in-order
queue never parks edge matmuls behind phase-1 work):
  1. h window w: xT chunks DMA'd in, 4 matmuls per [128,512] PSUM bank
     (lhsT = xT chunk, rhs = W, both bf16), one ACT copy -> bf16 staging,
     one DMA per 512 rows into h[w] DRAM.
  2. Edge pass window w: dma_gather h[col] rows (4 SWDGE queues rotating,
     32-tile chunks, 8 buffers in flight; ~2.9-3.4 ns/row isolated, ~4
     ns/row with concurrent bulk traffic), stream S tiles via HWDGE,
     PSUM-accumulate out_blk += S^T @ G per dest block (81 ns/matmul).
     Window-0 partials park in acc_sb; window-1 chains seed from them via
     an ACT SBUF->PSUM preload (start=False accumulation).
  3. Deferred epilogue per block: +bias, LayerNorm (ACT accum_out for
     var), gamma/beta, LeakyReLU via scalar_tensor_tensor max(x, 0.2x),
     DMA out.
"""

import sys

if "/opt/trn_rl_repo" not in sys.path:
    sys.path.insert(0, "/opt/trn_rl_repo")

import numpy as np
import ml_dtypes

from concourse import bacc, tile, mybir
from concourse.bass_utils import run_bass_kernel_spmd

# ---- problem constants (hardcoded per the harness contract) ----
N = 40000
E = 640000
D = 128
C = 8                # cores
NPC = 5000           # dest nodes per core
NB = 40              # dest blocks of 128 per core
NPCP = NB * 128      # 5120 padded dest rows per core
SRC_PAD = C * NPCP   # 40960 padded source rows
# gather windows (int16 index limit caps a window at 32768 rows); symmetric
# 20480/20480 measured faster than a small-A asymmetric split
WIN_A = 20480
WSZ = [WIN_A, SRC_PAD - WIN_A]
W0 = [0, WIN_A]
LN_EPS = 1e-5
LEAKY_ALPHA = 0.2

# ---- tunables ----
SKIP = frozenset()   # ablation flags: p1, g, sb, mm, epi
CH = 32              # gather chunk size in 128-edge tiles (24/12 and 64/4
NQ = 4               # variants measured slower; 32/8 is the sweet spot)
GBUFS = 8            # gather buffers in flight (2/queue; 10 and 12 measured
                     # identical-to-worse — the gather is not slot-starved)
XCH = 16             # node tiles per xT chunk DMA (2048 nodes)
PST = 4              # node tiles per P1 PSUM group ([128,512] bank)

_f32 = mybir.dt.float32
_bf16 = mybir.dt.bfloat16
_i16 = mybir.dt.int16


def _preprocess(edge_index, edge_weight):
    """Host: symmetric normalization, edge routing/grouping, shared padded
    schedule, per-core tile-layout tables (ewt/doft/idx16)."""
    row = np.asarray(edge_index[0], dtype=np.int64)
    col = np.asarray(edge_index[1], dtype=np.int64)
    ew = np.asarray(edge_weight, dtype=np.float32)
    deg = np.bincount(row, weights=ew, minlength=N)
    dis = np.where(deg > 0, deg ** -0.5, 0.0)
    ew = (dis[row] * ew * dis[col]).astype(np.float32)

    core = row // NPC
    lid = row - core * NPC
    win = (col >= WIN_A).astype(np.int64)     # source window 0|1
    crel = (col - win * WIN_A).astype(np.int16)  # 0..32767

    # Dest rows are assigned to blocks by BALANCED bin-packing (not lid>>7):
    # block membership within a core's output is free as long as the host
    # un-permutes afterwards, and balancing both windows' per-block edge
    # counts under 1024 gives every (core,window,block) group 8 tiles
    # instead of a variance-padded 9 -- ~8% fewer gathered rows.
    # Cap regular blocks at 1024 edges/window (8 tiles) and concentrate the
    # overflow (core-windows with > 40*1024 edges total) into KO designated
    # blocks per window, aligned across cores (w0: low block ids, w1: high)
    # so only those groups pay a 9th tile in the shared max-over-cores
    # schedule.  Plain balancing would smear the overload over all 40.
    KO = 4
    blk_of = np.zeros((C, NPC), np.int64)
    pos_of = np.zeros((C, NPC), np.int64)
    cap0 = np.where(np.arange(NB) < KO, 1152.0, 1024.0)
    cap1 = np.where(np.arange(NB) >= NB - KO, 1152.0, 1024.0)
    for c in range(C):
        m = core == c
        lm, wm = lid[m], win[m]
        na = np.bincount(lm[wm == 0], minlength=NPC).astype(np.float64)
        nb = np.bincount(lm[wm == 1], minlength=NPC).astype(np.float64)
        order_r = np.argsort(-(na + nb), kind="stable")
        sums = np.zeros((NB, 2))
        cnt = np.zeros(NB, np.int64)
        for r in order_r:
            f0 = sums[:, 0] + na[r]
            f1 = sums[:, 1] + nb[r]
            feas = (f0 <= cap0) & (f1 <= cap1) & (cnt < 128)
            if feas.any():
                load = np.maximum(f0 / cap0, f1 / cap1)
                load[~feas] = np.inf
            else:
                load = np.maximum(f0, f1) + (cnt >= 128) * 1e9
            b = int(np.argmin(load))
            blk_of[c, r] = b
            pos_of[c, r] = cnt[b]
            cnt[b] += 1
            sums[b, 0] += na[r]
            sums[b, 1] += nb[r]
    blk = blk_of[core, lid]
    off = pos_of[core, lid].astype(np.float32)
    # outmap[c, 128*b + pos] = original lid (or -1 for pad slots)
    outmap = np.full((C, NPCP), -1, np.int64)
    for c in range(C):
        outmap[c, 128 * blk_of[c] + pos_of[c]] = np.arange(NPC)
    global _OUTMAP
    _OUTMAP = outmap

    # group key per edge: (core, window, block)
    key = (core * 2 + win) * NB + blk
    counts = np.bincount(key, minlength=C * 2 * NB).reshape(C, 2, NB)
    # shared schedule: tiles per (window, block) = max over cores, rounded up
    P = 128 * np.ceil(counts.max(axis=0) / 128.0).astype(np.int64)  # [2, NB]
    for b in range(NB):
        if P[0, b] + P[1, b] == 0:
            P[1, b] = 128  # at least one (all-dummy) tile per block

    Ppad = int(P.sum())
    T = Ppad // 128
    gstart = np.concatenate([[0], np.cumsum(P.reshape(-1))])[:-1].reshape(2, NB)

    colr_a = np.zeros((C, Ppad), np.int16)
    ew_a = np.zeros((C, Ppad), np.float32)
    dof_a = np.zeros((C, Ppad), np.float32)

    # sort by (group, source row): order within a group is free, ascending
    # source addresses give the gather slightly better DRAM locality
    order = np.lexsort((crel, key))
    key_s = key[order]
    grp_sizes = counts.reshape(-1)
    grp_off = np.concatenate([[0], np.cumsum(grp_sizes)])[:-1]
    within = np.arange(len(key_s)) - grp_off[key_s]
    c_s = key_s // (2 * NB)
    wb_s = key_s % (2 * NB)
    dest = gstart.reshape(-1)[wb_s] + within
    colr_a[c_s, dest] = crel[order]
    ew_a[c_s, dest] = ew[order]
    dof_a[c_s, dest] = off[order]

    # S tiles precomputed on HOST (static edge data): S[e, d] = norm_e if
    # dest_off_e == d else 0.  Layout [C, 128 edge, T*128] bf16 so tile t is
    # the column slice [:, 128t:128(t+1)] — streamed to SBUF and fed to the
    # PE as lhsT directly.  This keeps DVE silent during the edge pass: DVE
    # work would lock GPSIMD out of the shared SBUF port it needs to write
    # SWDGE gather descriptors, starving the gather DMA.
    st = np.zeros((C, 128, T * 128), ml_dtypes.bfloat16)
    cc = c_s
    tt = dest // 128
    ee = (dest % 128).astype(np.int64)
    dd = dof_a[c_s, dest].astype(np.int64)
    st[cc, ee, tt * 128 + dd] = ew[order].astype(ml_dtypes.bfloat16)
    # dma_gather index layout: idx i of a tile at [i%16, i//16], replicated x8
    A = colr_a.reshape(C, T, 8, 16).transpose(0, 1, 3, 2)                      # [C,T,16,8]
    idx16 = np.tile(A, (1, 1, 8, 1)).transpose(0, 2, 1, 3).reshape(C, 128, 8 * T)
    return P, T, st, np.ascontiguousarray(idx16)


def _schedule(P):
    """Static tile schedule shared by all cores."""
    P = np.asarray(P)
    tiles = []  # global tile idx -> (w, b)
    for w in (0, 1):
        for b in range(NB):
            for _ in range(int(P[w, b]) // 128):
                tiles.append((w, b))
    T = len(tiles)
    t0w = [0, int(P[0].sum()) // 128]
    Tw = [t0w[1], T - t0w[1]]
    return tiles, t0w, Tw


def _build(P, T, tiles, t0w, Tw, sim_single_core=False, reps=1):
    del sim_single_core  # v2 has no collectives; kept for test.py compat
    nc = bacc.Bacc("TRN2", target_bir_lowering=False, debug=False,
                   enable_asserts=True, num_devices=1,
                   num_swdge_queues=NQ, dynamic_dma_scratch_size=65536)

    xt_in = nc.dram_tensor("xt", [128, SRC_PAD], _bf16, kind="ExternalInput").ap()
    w_in = nc.dram_tensor("w", [D, D], _bf16, kind="ExternalInput").ap()
    bias_in = nc.dram_tensor("bias", [1, D], _f32, kind="ExternalInput").ap()
    gamma_in = nc.dram_tensor("gamma", [1, D], _f32, kind="ExternalInput").ap()
    beta_in = nc.dram_tensor("beta", [1, D], _f32, kind="ExternalInput").ap()
    st_in = nc.dram_tensor("st", [128, 128 * T], _bf16, kind="ExternalInput").ap()
    idx_in = nc.dram_tensor("idx16", [128, 8 * T], _i16, kind="ExternalInput").ap()
    out_d = nc.dram_tensor("out", [NPCP, D], _f32, kind="ExternalOutput").ap()

    eq = mybir.AluOpType.is_equal
    mul = mybir.AluOpType.mult
    add = mybir.AluOpType.add
    AF = mybir.ActivationFunctionType

    # per-block window runs: (first_tile, last_tile) or None
    runs = [[None, None] for _ in range(NB)]
    for t, (w, b) in enumerate(tiles):
        if runs[b][w] is None:
            runs[b][w] = [t, t]
        else:
            runs[b][w][1] = t

    with tile.TileContext(nc) as tc:
        with (
            tc.tile_pool(name="const", bufs=1) as cp,
            tc.tile_pool(name="resident", bufs=1) as rp,
            tc.tile_pool(name="dram", bufs=1, space="DRAM") as dp,
        ):
            ones_row = cp.tile([1, 128], _f32)
            nc.vector.memset(ones_row[:], 1.0)
            eps_col = cp.tile([128, 1], _f32)
            nc.vector.memset(eps_col[:], float(LN_EPS))
            w_sb = cp.tile([128, 128], _bf16)
            nc.sync.dma_start(w_sb[:], w_in)

            # broadcast bias/gamma/beta rows to all 128 partitions via matmul
            bias_bc = cp.tile([128, 128], _f32)
            gamma_bc = cp.tile([128, 128], _f32)
            beta_bc = cp.tile([128, 128], _f32)
            with tc.tile_pool(name="bc", bufs=1) as bcp, \
                 tc.tile_pool(name="bcps", bufs=1, space="PSUM") as bcps:
                for src_ap, dst in ((bias_in, bias_bc), (gamma_in, gamma_bc),
                                    (beta_in, beta_bc)):
                    r = bcp.tile([1, 128], _f32, tag="bcrow")
                    nc.sync.dma_start(r[:], src_ap)
                    ps = bcps.tile([128, 128], _f32, tag="bcps")
                    nc.tensor.matmul(ps[:], lhsT=ones_row[:], rhs=r[:],
                                     start=True, stop=True)
                    nc.scalar.copy(dst[:], ps[:])

            idx_sb = rp.tile([128, 8 * T], _i16)
            nc.sync.dma_start(idx_sb[:], idx_in)

            acc_sb = rp.tile([128, NB, 128], _f32)  # window-0 partial sums

            h_a = dp.tile([WSZ[0], D], _bf16)
            h_b = dp.tile([WSZ[1], D], _bf16)
            h_d = [h_a, h_b]

            dmaeng = [nc.sync, nc.scalar]  # HWDGE streams for plain DMAs

            def _phases():
                # ---------------- Phase 1: h[w] = bf16(x @ W) ----------------
                def p1_window(w):
                    if "p1" in SKIP:
                        # ablation: fill h with arbitrary bytes in one DMA so
                        # gather reads aren't reads-before-any-write
                        hv = h_d[w][:].rearrange("(t p) d -> p t d", p=128)
                        src = xt_in[:, W0[w]:W0[w] + WSZ[w]].rearrange(
                            "p (t d) -> p t d", d=128)
                        nc.sync.dma_start(hv, src)
                        return
                    nchunks = WSZ[w] // (XCH * 128)
                    for ci in range(nchunks):
                        base = W0[w] + ci * XCH * 128
                        xc = p1x.tile([128, XCH * 128], _bf16, tag="xc")
                        dmaeng[ci % 2].dma_start(
                            xc[:], xt_in[:, base:base + XCH * 128])
                        for g in range(XCH // PST):
                            ps = p1ps.tile([128, PST * 128], _f32, tag="hps")
                            for j in range(PST):
                                k = (g * PST + j) * 128
                                nc.tensor.matmul(
                                    ps[:, j * 128:(j + 1) * 128],
                                    lhsT=xc[:, k:k + 128], rhs=w_sb[:],
                                    start=True, stop=True)
                            hst = p1h.tile([128, PST, 128], _bf16, tag="hst")
                            nc.scalar.copy(hst[:], ps[:])
                            r0 = ci * XCH * 128 + g * PST * 128
                            dst = h_d[w][r0:r0 + PST * 128, :].rearrange(
                                "(t p) d -> p t d", p=128)
                            dmaeng[(ci + g) % 2].dma_start(dst, hst[:])

                # ---------------- Edge pass: gather + scatter matmuls --------
                def epilogue(b, bias_src):
                    # gated on bias_src (copied only after the final gather):
                    # epilogue DVE ops would otherwise run mid-pass and lock
                    # GPSIMD out of SWDGE gather-descriptor generation
                    if "epi" in SKIP:
                        return
                    t4 = ep.tile([128, 128], _f32, tag="e_t4")
                    nc.vector.tensor_tensor(t4[:], acc_sb[:, b, :],
                                            bias_src[:], op=add)
                    nsum = ep.tile([128, 1], _f32, tag="e_ns")
                    nc.vector.tensor_reduce(nsum[:], t4[:],
                                            axis=mybir.AxisListType.X,
                                            op=add, negate=True)
                    nmean = ep.tile([128, 1], _f32, tag="e_nm")
                    nc.scalar.mul(nmean[:], nsum[:], 1.0 / 128.0)
                    t5 = ep.tile([128, 128], _f32, tag="e_t5")
                    nc.scalar.activation(t5[:], t4[:], AF.Identity,
                                         bias=nmean[:], scale=1.0)
                    sq = ep.tile([128, 128], _f32, tag="e_sq")
                    vsum = ep.tile([128, 1], _f32, tag="e_vs")
                    nc.scalar.activation(sq[:], t5[:], AF.Square,
                                         accum_out=vsum[:])
                    sd = ep.tile([128, 1], _f32, tag="e_sd")
                    nc.scalar.activation(sd[:], vsum[:], AF.Sqrt,
                                         scale=1.0 / 128.0, bias=eps_col[:])
                    rstd = ep.tile([128, 1], _f32, tag="e_rs")
                    nc.vector.reciprocal(rstd[:], sd[:])
                    t6 = ep.tile([128, 128], _f32, tag="e_t6")
                    nc.vector.scalar_tensor_tensor(t6[:], t5[:], rstd[:],
                                                   gamma_bc[:], op0=mul, op1=mul)
                    nc.vector.tensor_tensor(t6[:], t6[:], beta_bc[:], op=add)
                    osb = ep.tile([128, 128], _f32, tag="e_o")
                    nc.vector.scalar_tensor_tensor(osb[:], t6[:],
                                                   float(LEAKY_ALPHA), t6[:],
                                                   op0=mul,
                                                   op1=mybir.AluOpType.max)
                    nc.sync.dma_start(out_d[b * 128:(b + 1) * 128, :], osb[:])

                gcall = [0]

                def edge_window(w):
                    win_ap = h_d[w][:]
                    nt_left = Tw[w]
                    t0 = t0w[w]
                    cur_ps = None
                    while nt_left > 0:
                        nt = min(CH, nt_left)
                        gbuf = gbp.tile([128, CH, 128], _bf16, tag="g")
                        if "gs" in SKIP:
                            # ablation: same bytes, sequential HWDGE DMA
                            src = h_d[w][:nt * 128, :].rearrange(
                                "(t p) d -> p t d", p=128)
                            dmaeng[gcall[0] % 2].dma_start(gbuf[:, :nt, :], src)
                            gcall[0] += 1
                        elif "g" in SKIP:
                            nc.vector.memset(gbuf[:], 0.0)
                        else:
                            nc.gpsimd.dma_gather(
                                out_ap=gbuf[:, :nt, :], in_ap=win_ap,
                                idxs_ap=idx_sb[:, 8 * t0:8 * (t0 + nt)],
                                num_idxs=128 * nt, num_idxs_reg=128 * nt,
                                elem_size=128, single_packet=False,
                                queue_num=gcall[0] % NQ)
                            gcall[0] += 1
                        if "sb" not in SKIP:  # "sb" skip implies "mm" skip
                            st_sb = stp.tile([128, CH * 128], _bf16, tag="st")
                            dmaeng[gcall[0] % 2].dma_start(
                                st_sb[:, :nt * 128],
                                st_in[:, 128 * t0:128 * (t0 + nt)])
                        for s_i in range(nt):
                            t = t0 + s_i
                            b = tiles[t][1]
                            first = runs[b][w][0] == t
                            last = runs[b][w][1] == t
                            preload = w == 1 and runs[b][0] is not None
                            if first:
                                cur_ps = pbps.tile([128, 128], _f32, tag="blk")
                                if preload and "mm" not in SKIP:
                                    # seed the accumulation with the window-0
                                    # partial so no separate add is needed
                                    nc.scalar.copy(cur_ps[:], acc_sb[:, b, :])
                                if "mm" in SKIP:
                                    nc.vector.memset(cur_ps[:], 0.0)
                            if "mm" not in SKIP:
                                nc.tensor.matmul(
                                    cur_ps[:],
                                    lhsT=st_sb[:, 128 * s_i:128 * (s_i + 1)],
                                    rhs=gbuf[:, s_i, :],
                                    start=first and not preload, stop=last)
                            if last:
                                nc.scalar.copy(acc_sb[:, b, :], cur_ps[:])
                                cur_ps = None
                        t0 += nt
                        nt_left -= nt

                with tc.tile_pool(name="p1x", bufs=2) as p1x, \
                     tc.tile_pool(name="p1h", bufs=3) as p1h, \
                     tc.tile_pool(name="p1ps", bufs=3, space="PSUM") as p1ps, \
                     tc.tile_pool(name="stp", bufs=3) as stp, \
                     tc.tile_pool(name="gb", bufs=GBUFS) as gbp, \
                     tc.tile_pool(name="pbps", bufs=4, space="PSUM") as pbps, \
                     tc.tile_pool(name="ep", bufs=2) as ep:
                    # p1(0), edge(0), p1(1), edge(1): keeps PE's in-order
                    # queue from parking edge-A matmuls behind P1-B, which
                    # would stall gather-A once the gather buffers fill
                    p1_window(0)
                    edge_window(0)
                    p1_window(1)
                    edge_window(1)
                    # bias_gated is written after the last edge-pass psum
                    # copy in ACT program order; every epilogue reads it, so
                    # no epilogue DVE op can start before the gathers end
                    bias_gated = ep.tile([128, 128], _f32, tag="e_bg")
                    nc.scalar.copy(bias_gated[:], bias_bc[:])
                    for b in range(NB):
                        epilogue(b, bias_gated)

            if reps == 1:
                _phases()
            else:
                with tc.For_i(0, reps, 1):
                    _phases()

    nc.compile()
    return nc


def _core_maps(x, weight, bias, gamma, beta, st, idx16):
    """Per-core input maps (shared by kernel() and test.py)."""
    xt = np.zeros((128, SRC_PAD), ml_dtypes.bfloat16)
    xt[:, :N] = np.asarray(x, np.float32).T.astype(ml_dtypes.bfloat16)
    wb = np.asarray(weight, np.float32).astype(ml_dtypes.bfloat16)
    bias = np.asarray(bias, np.float32).reshape(1, D)
    gamma = np.asarray(gamma, np.float32).reshape(1, D)
    beta = np.asarray(beta, np.float32).reshape(1, D)
    maps = []
    for c in range(C):
        maps.append({
            "xt": xt, "w": wb, "bias": bias, "gamma": gamma, "beta": beta,
            "st": np.ascontiguousarray(st[c]),
            "idx16": np.ascontiguousarray(idx16[c]),
        })
    return maps


_CACHE = {}
_OUTMAP = None


def _get_compiled(edge_index, edge_weight):
    P, T, st, idx16 = _preprocess(edge_index, edge_weight)
    key = P.tobytes()
    if key not in _CACHE:
        tiles, t0w, Tw = _schedule(P)
        _CACHE[key] = _build(P, T, tiles, t0w, Tw)
    return _CACHE[key], st, idx16


def kernel(x, edge_index, edge_weight, weight, adaptive_weight, bias,
           ln_gamma, ln_beta):
    nc, st, idx16 = _get_compiled(edge_index, edge_weight)
    in_maps = _core_maps(x, weight, bias, ln_gamma, ln_beta, st, idx16)
    res = run_bass_kernel_spmd(nc, in_maps, core_ids=list(range(C)))
    out = np.empty((N, D), np.float32)
    for c in range(C):
        oc = np.asarray(res.results[c]["out"])
        m = _OUTMAP[c]
        valid = m >= 0
        out[c * NPC + m[valid]] = oc[valid]
    return out


# revision 42
# speedup vs baseline: 1.2447x; 1.0208x over previous
"""AGCNConv (GNN message passing) distributed Bass kernel for 8 TRN2 NeuronCores.

Reference math:
    h   = x @ W
    aew = edge_weight * sigmoid(adaptive_weight)
    deg = segment_sum(aew, row);  dis = where(deg>0, deg^-1/2, 0)
    out = segment_sum(h[col] * (dis[row]*aew*dis[col])[:,None], row)
    out = LeakyReLU(LayerNorm(out + bias))

Key identities exploited:
  * The global factor s = sigmoid(adaptive_weight) cancels in the symmetric
    normalization, so adaptive_weight is unused.
  * norm_e = dis[row]*ew*dis[col] depends only on edge data -> computed on
    the HOST in _preprocess; the device never computes degrees.

v3 design (no collectives): every core receives the FULL transposed node
matrix xT (bf16, replicated input) and redundantly computes h = x @ W for
all 40960 padded source rows, writing bf16 h to its local DRAM in two
20480-row windows (the int16 gather-index limit). Dest nodes are sharded
8 ways (5120 padded rows per core); edges are routed to their dest core
and grouped by (source window, dest block of 128), sorted by source row
within each group (gather locality), padded to 128-edge tiles with a
schedule shared across all cores (SPMD: one program, per-core tables).

The central hardware constraint (measured): DVE and GPSIMD arbitrate an
EXCLUSIVE shared SBUF port, and SWDGE gather-descriptor generation runs
on GPSIMD — any steady-state DVE work starves the gather DMA (kernel went
866us -> 560us when DVE left the loop). Hence:
  * S matrices (S[e,d] = norm_e * onehot(dest_off_e)) are built on the
    HOST and streamed via HWDGE as bf16 tiles — no DVE S-build.
  * All epilogue DVE ops are data-gated (via bias_gated) on the LAST
    gather so they cannot run mid-pass.

Device pipeline per core (emitted p1A, edgeA, p1B, edgeB so PE's in-order
queue never parks edge matmuls behind phase-1 work):
  1. h window w: xT chunks DMA'd in, 4 matmuls per [128,512] PSUM bank
     (lhsT = xT chunk, rhs = W, both bf16), one ACT copy -> bf16 staging,
     one DMA per 512 rows into h[w] DRAM.
  2. Edge pass window w: dma_gather h[col] rows (4 SWDGE queues rotating,
     32-tile chunks, 8 buffers in flight; ~2.9-3.4 ns/row isolated, ~4
     ns/row with concurrent bulk traffic), stream S tiles via HWDGE,
     PSUM-accumulate out_blk += S^T @ G per dest block (81 ns/matmul).
     Window-0 partials park in acc_sb; window-1 chains seed from them via
     an ACT SBUF->PSUM preload (start=False accumulation).
  3. Deferred epilogue per block: +bias, LayerNorm (ACT accum_out for
     var), gamma/beta, LeakyReLU via scalar_tensor_tensor max(x, 0.2x),
     DMA out.

Measured history: 882us baseline -> 866 (collectives removed) -> 560
(host S stream) -> 515 (PE reorder + CH32/G8 + gated epilogue) -> 487
(balanced blocks) -> 446 (capped-overflow packing, T=648).

Remaining known headroom for future work (~30-50us total):
  * T=648 vs lower bound 629 (= sum over windows of the worst core's
    ceil(total/128)); window boundary 19456 balances the windows and a
    per-window KO could reach ~638 (~3-7us).
  * The epilogue gate (bias_gated) is conservative: SWDGE desc-gen for
    the LAST gather call actually completes ~8 chunks early (Pool runs
    ahead until gather-buffer slots block), so the gate could hang off a
    mid-window-1 acc copy instead of the final one, reclaiming most of
    the ~25us tail. Needs timing validation - if Pool runahead is
    shallower than GBUFS suggests, mid-pass DVE locks return.
  * Single contiguous h with per-tile base offsets (0/8192) removes the
    window split and the acc_sb park/preload machinery entirely (~20us
    net after a longer prefix); full preprocessing/edge-pass rewrite.
  * Per-core dynamic gather counts via num_idxs_reg (negative padding
    indices) would cut ~8% of gather rows but risks 0*NaN poisoning
    from uninitialized gather-buffer SBUF on first use.
"""

import sys

if "/opt/trn_rl_repo" not in sys.path:
    sys.path.insert(0, "/opt/trn_rl_repo")

import numpy as np
import ml_dtypes

from concourse import bacc, tile, mybir
from concourse.bass_utils import run_bass_kernel_spmd

# ---- problem constants (hardcoded per the harness contract) ----
N = 40000
E = 640000
D = 128
C = 8                # cores
NPC = 5000           # dest nodes per core
NB = 40              # dest blocks of 128 per core
NPCP = NB * 128      # 5120 padded dest rows per core
SRC_PAD = C * NPCP   # 40960 padded source rows
# gather windows (int16 index limit caps a window at 32768 rows); symmetric
# 20480/20480 measured faster than a small-A asymmetric split
WIN_A = 20480
WSZ = [WIN_A, SRC_PAD - WIN_A]
W0 = [0, WIN_A]
LN_EPS = 1e-5
LEAKY_ALPHA = 0.2

# ---- tunables ----
SKIP = frozenset()   # ablation flags: p1, g, sb, mm, epi
CH = 32              # gather chunk size in 128-edge tiles (24/12 and 64/4
NQ = 4               # variants measured slower; 32/8 is the sweet spot)
GBUFS = 8            # gather buffers in flight (2/queue; 10 and 12 measured
                     # identical-to-worse — the gather is not slot-starved)
XCH = 16             # node tiles per xT chunk DMA (2048 nodes)
PST = 4              # node tiles per P1 PSUM group ([128,512] bank)

_f32 = mybir.dt.float32
_bf16 = mybir.dt.bfloat16
_i16 = mybir.dt.int16


def _preprocess(edge_index, edge_weight):
    """Host: symmetric normalization, edge routing/grouping, shared padded
    schedule, per-core tile-layout tables (ewt/doft/idx16)."""
    row = np.asarray(edge_index[0], dtype=np.int64)
    col = np.asarray(edge_index[1], dtype=np.int64)
    ew = np.asarray(edge_weight, dtype=np.float32)
    deg = np.bincount(row, weights=ew, minlength=N)
    dis = np.where(deg > 0, deg ** -0.5, 0.0)
    ew = (dis[row] * ew * dis[col]).astype(np.float32)

    core = row // NPC
    lid = row - core * NPC
    win = (col >= WIN_A).astype(np.int64)     # source window 0|1
    crel = (col - win * WIN_A).astype(np.int16)  # 0..32767

    # Dest rows are assigned to blocks by BALANCED bin-packing (not lid>>7):
    # block membership within a core's output is free as long as the host
    # un-permutes afterwards, and balancing both windows' per-block edge
    # counts under 1024 gives every (core,window,block) group 8 tiles
    # instead of a variance-padded 9 -- ~8% fewer gathered rows.
    # Cap regular blocks at 1024 edges/window (8 tiles) and concentrate the
    # overflow (core-windows with > 40*1024 edges total) into KO designated
    # blocks per window, aligned across cores (w0: low block ids, w1: high)
    # so only those groups pay a 9th tile in the shared max-over-cores
    # schedule.  Plain balancing would smear the overload over all 40.
    KO = 4
    blk_of = np.zeros((C, NPC), np.int64)
    pos_of = np.zeros((C, NPC), np.int64)
    cap0 = np.where(np.arange(NB) < KO, 1152.0, 1024.0)
    cap1 = np.where(np.arange(NB) >= NB - KO, 1152.0, 1024.0)
    for c in range(C):
        m = core == c
        lm, wm = lid[m], win[m]
        na = np.bincount(lm[wm == 0], minlength=NPC).astype(np.float64)
        nb = np.bincount(lm[wm == 1], minlength=NPC).astype(np.float64)
        order_r = np.argsort(-(na + nb), kind="stable")
        sums = np.zeros((NB, 2))
        cnt = np.zeros(NB, np.int64)
        for r in order_r:
            f0 = sums[:, 0] + na[r]
            f1 = sums[:, 1] + nb[r]
            feas = (f0 <= cap0) & (f1 <= cap1) & (cnt < 128)
            if feas.any():
                load = np.maximum(f0 / cap0, f1 / cap1)
                load[~feas] = np.inf
            else:
                load = np.maximum(f0, f1) + (cnt >= 128) * 1e9
            b = int(np.argmin(load))
            blk_of[c, r] = b
            pos_of[c, r] = cnt[b]
            cnt[b] += 1
            sums[b, 0] += na[r]
            sums[b, 1] += nb[r]
    blk = blk_of[core, lid]
    off = pos_of[core, lid].astype(np.float32)
    # outmap[c, 128*b + pos] = original lid (or -1 for pad slots)
    outmap = np.full((C, NPCP), -1, np.int64)
    for c in range(C):
        outmap[c, 128 * blk_of[c] + pos_of[c]] = np.arange(NPC)
    global _OUTMAP
    _OUTMAP = outmap

    # group key per edge: (core, window, block)
    key = (core * 2 + win) * NB + blk
    counts = np.bincount(key, minlength=C * 2 * NB).reshape(C, 2, NB)
    # shared schedule: tiles per (window, block) = max over cores, rounded up
    P = 128 * np.ceil(counts.max(axis=0) / 128.0).astype(np.int64)  # [2, NB]
    for b in range(NB):
        if P[0, b] + P[1, b] == 0:
            P[1, b] = 128  # at least one (all-dummy) tile per block

    Ppad = int(P.sum())
    T = Ppad // 128
    gstart = np.concatenate([[0], np.cumsum(P.reshape(-1))])[:-1].reshape(2, NB)

    colr_a = np.zeros((C, Ppad), np.int16)
    ew_a = np.zeros((C, Ppad), np.float32)
    dof_a = np.zeros((C, Ppad), np.float32)

    # sort by (group, source row): order within a group is free, ascending
    # source addresses give the gather slightly better DRAM locality
    order = np.lexsort((crel, key))
    key_s = key[order]
    grp_sizes = counts.reshape(-1)
    grp_off = np.concatenate([[0], np.cumsum(grp_sizes)])[:-1]
    within = np.arange(len(key_s)) - grp_off[key_s]
    c_s = key_s // (2 * NB)
    wb_s = key_s % (2 * NB)
    dest = gstart.reshape(-1)[wb_s] + within
    colr_a[c_s, dest] = crel[order]
    ew_a[c_s, dest] = ew[order]
    dof_a[c_s, dest] = off[order]

    # S tiles precomputed on HOST (static edge data): S[e, d] = norm_e if
    # dest_off_e == d else 0.  Layout [C, 128 edge, T*128] bf16 so tile t is
    # the column slice [:, 128t:128(t+1)] — streamed to SBUF and fed to the
    # PE as lhsT directly.  This keeps DVE silent during the edge pass: DVE
    # work would lock GPSIMD out of the shared SBUF port it needs to write
    # SWDGE gather descriptors, starving the gather DMA.
    st = np.zeros((C, 128, T * 128), ml_dtypes.bfloat16)
    cc = c_s
    tt = dest // 128
    ee = (dest % 128).astype(np.int64)
    dd = dof_a[c_s, dest].astype(np.int64)
    st[cc, ee, tt * 128 + dd] = ew[order].astype(ml_dtypes.bfloat16)
    # dma_gather index layout: idx i of a tile at [i%16, i//16], replicated x8
    A = colr_a.reshape(C, T, 8, 16).transpose(0, 1, 3, 2)                      # [C,T,16,8]
    idx16 = np.tile(A, (1, 1, 8, 1)).transpose(0, 2, 1, 3).reshape(C, 128, 8 * T)
    return P, T, st, np.ascontiguousarray(idx16)


def _schedule(P):
    """Static tile schedule shared by all cores."""
    P = np.asarray(P)
    tiles = []  # global tile idx -> (w, b)
    for w in (0, 1):
        for b in range(NB):
            for _ in range(int(P[w, b]) // 128):
                tiles.append((w, b))
    T = len(tiles)
    t0w = [0, int(P[0].sum()) // 128]
    Tw = [t0w[1], T - t0w[1]]
    return tiles, t0w, Tw


def _build(P, T, tiles, t0w, Tw, sim_single_core=False, reps=1):
    del sim_single_core  # v2 has no collectives; kept for test.py compat
    nc = bacc.Bacc("TRN2", target_bir_lowering=False, debug=False,
                   enable_asserts=True, num_devices=1,
                   num_swdge_queues=NQ, dynamic_dma_scratch_size=65536)

    xt_in = nc.dram_tensor("xt", [128, SRC_PAD], _bf16, kind="ExternalInput").ap()
    w_in = nc.dram_tensor("w", [D, D], _bf16, kind="ExternalInput").ap()
    bias_in = nc.dram_tensor("bias", [1, D], _f32, kind="ExternalInput").ap()
    gamma_in = nc.dram_tensor("gamma", [1, D], _f32, kind="ExternalInput").ap()
    beta_in = nc.dram_tensor("beta", [1, D], _f32, kind="ExternalInput").ap()
    st_in = nc.dram_tensor("st", [128, 128 * T], _bf16, kind="ExternalInput").ap()
    idx_in = nc.dram_tensor("idx16", [128, 8 * T], _i16, kind="ExternalInput").ap()
    out_d = nc.dram_tensor("out", [NPCP, D], _f32, kind="ExternalOutput").ap()

    eq = mybir.AluOpType.is_equal
    mul = mybir.AluOpType.mult
    add = mybir.AluOpType.add
    AF = mybir.ActivationFunctionType

    # per-block window runs: (first_tile, last_tile) or None
    runs = [[None, None] for _ in range(NB)]
    for t, (w, b) in enumerate(tiles):
        if runs[b][w] is None:
            runs[b][w] = [t, t]
        else:
            runs[b][w][1] = t

    with tile.TileContext(nc) as tc:
        with (
            tc.tile_pool(name="const", bufs=1) as cp,
            tc.tile_pool(name="resident", bufs=1) as rp,
            tc.tile_pool(name="dram", bufs=1, space="DRAM") as dp,
        ):
            ones_row = cp.tile([1, 128], _f32)
            nc.vector.memset(ones_row[:], 1.0)
            eps_col = cp.tile([128, 1], _f32)
            nc.vector.memset(eps_col[:], float(LN_EPS))
            w_sb = cp.tile([128, 128], _bf16)
            nc.sync.dma_start(w_sb[:], w_in)

            # broadcast bias/gamma/beta rows to all 128 partitions via matmul
            bias_bc = cp.tile([128, 128], _f32)
            gamma_bc = cp.tile([128, 128], _f32)
            beta_bc = cp.tile([128, 128], _f32)
            with tc.tile_pool(name="bc", bufs=1) as bcp, \
                 tc.tile_pool(name="bcps", bufs=1, space="PSUM") as bcps:
                for src_ap, dst in ((bias_in, bias_bc), (gamma_in, gamma_bc),
                                    (beta_in, beta_bc)):
                    r = bcp.tile([1, 128], _f32, tag="bcrow")
                    nc.sync.dma_start(r[:], src_ap)
                    ps = bcps.tile([128, 128], _f32, tag="bcps")
                    nc.tensor.matmul(ps[:], lhsT=ones_row[:], rhs=r[:],
                                     start=True, stop=True)
                    nc.scalar.copy(dst[:], ps[:])

            idx_sb = rp.tile([128, 8 * T], _i16)
            nc.sync.dma_start(idx_sb[:], idx_in)

            acc_sb = rp.tile([128, NB, 128], _f32)  # window-0 partial sums

            h_a = dp.tile([WSZ[0], D], _bf16)
            h_b = dp.tile([WSZ[1], D], _bf16)
            h_d = [h_a, h_b]

            dmaeng = [nc.sync, nc.scalar]  # HWDGE streams for plain DMAs

            def _phases():
                # ---------------- Phase 1: h[w] = bf16(x @ W) ----------------
                def p1_window(w):
                    if "p1" in SKIP:
                        # ablation: fill h with arbitrary bytes in one DMA so
                        # gather reads aren't reads-before-any-write
                        hv = h_d[w][:].rearrange("(t p) d -> p t d", p=128)
                        src = xt_in[:, W0[w]:W0[w] + WSZ[w]].rearrange(
                            "p (t d) -> p t d", d=128)
                        nc.sync.dma_start(hv, src)
                        return
                    nchunks = WSZ[w] // (XCH * 128)
                    for ci in range(nchunks):
                        base = W0[w] + ci * XCH * 128
                        xc = p1x.tile([128, XCH * 128], _bf16, tag="xc")
                        dmaeng[ci % 2].dma_start(
                            xc[:], xt_in[:, base:base + XCH * 128])
                        for g in range(XCH // PST):
                            ps = p1ps.tile([128, PST * 128], _f32, tag="hps")
                            for j in range(PST):
                                k = (g * PST + j) * 128
                                nc.tensor.matmul(
                                    ps[:, j * 128:(j + 1) * 128],
                                    lhsT=xc[:, k:k + 128], rhs=w_sb[:],
                                    start=True, stop=True)
                            hst = p1h.tile([128, PST, 128], _bf16, tag="hst")
                            nc.scalar.copy(hst[:], ps[:])
                            r0 = ci * XCH * 128 + g * PST * 128
                            dst = h_d[w][r0:r0 + PST * 128, :].rearrange(
                                "(t p) d -> p t d", p=128)
                            dmaeng[(ci + g) % 2].dma_start(dst, hst[:])

                # ---------------- Edge pass: gather + scatter matmuls --------
                def epilogue(b, bias_src):
                    # gated on bias_src (copied only after the final gather):
                    # epilogue DVE ops would otherwise run mid-pass and lock
                    # GPSIMD out of SWDGE gather-descriptor generation
                    if "epi" in SKIP:
                        return
                    t4 = ep.tile([128, 128], _f32, tag="e_t4")
                    nc.vector.tensor_tensor(t4[:], acc_sb[:, b, :],
                                            bias_src[:], op=add)
                    nsum = ep.tile([128, 1], _f32, tag="e_ns")
                    nc.vector.tensor_reduce(nsum[:], t4[:],
                                            axis=mybir.AxisListType.X,
                                            op=add, negate=True)
                    nmean = ep.tile([128, 1], _f32, tag="e_nm")
                    nc.scalar.mul(nmean[:], nsum[:], 1.0 / 128.0)
                    t5 = ep.tile([128, 128], _f32, tag="e_t5")
                    nc.scalar.activation(t5[:], t4[:], AF.Identity,
                                         bias=nmean[:], scale=1.0)
                    sq = ep.tile([128, 128], _f32, tag="e_sq")
                    vsum = ep.tile([128, 1], _f32, tag="e_vs")
                    nc.scalar.activation(sq[:], t5[:], AF.Square,
                                         accum_out=vsum[:])
                    sd = ep.tile([128, 1], _f32, tag="e_sd")
                    nc.scalar.activation(sd[:], vsum[:], AF.Sqrt,
                                         scale=1.0 / 128.0, bias=eps_col[:])
                    rstd = ep.tile([128, 1], _f32, tag="e_rs")
                    nc.vector.reciprocal(rstd[:], sd[:])
                    t6 = ep.tile([128, 128], _f32, tag="e_t6")
                    nc.vector.scalar_tensor_tensor(t6[:], t5[:], rstd[:],
                                                   gamma_bc[:], op0=mul, op1=mul)
                    nc.vector.tensor_tensor(t6[:], t6[:], beta_bc[:], op=add)
                    osb = ep.tile([128, 128], _f32, tag="e_o")
                    nc.vector.scalar_tensor_tensor(osb[:], t6[:],
                                                   float(LEAKY_ALPHA), t6[:],
                                                   op0=mul,
                                                   op1=mybir.AluOpType.max)
                    nc.sync.dma_start(out_d[b * 128:(b + 1) * 128, :], osb[:])

                gcall = [0]

                def edge_window(w):
                    win_ap = h_d[w][:]
                    nt_left = Tw[w]
                    t0 = t0w[w]
                    cur_ps = None
                    while nt_left > 0:
                        nt = min(CH, nt_left)
                        gbuf = gbp.tile([128, CH, 128], _bf16, tag="g")
                        if "gs" in SKIP:
                            # ablation: same bytes, sequential HWDGE DMA
                            src = h_d[w][:nt * 128, :].rearrange(
                                "(t p) d -> p t d", p=128)
                            dmaeng[gcall[0] % 2].dma_start(gbuf[:, :nt, :], src)
                            gcall[0] += 1
                        elif "g" in SKIP:
                            nc.vector.memset(gbuf[:], 0.0)
                        else:
                            nc.gpsimd.dma_gather(
                                out_ap=gbuf[:, :nt, :], in_ap=win_ap,
                                idxs_ap=idx_sb[:, 8 * t0:8 * (t0 + nt)],
                                num_idxs=128 * nt, num_idxs_reg=128 * nt,
                                elem_size=128, single_packet=False,
                                queue_num=gcall[0] % NQ)
                            gcall[0] += 1
                        if "sb" not in SKIP:  # "sb" skip implies "mm" skip
                            st_sb = stp.tile([128, CH * 128], _bf16, tag="st")
                            dmaeng[gcall[0] % 2].dma_start(
                                st_sb[:, :nt * 128],
                                st_in[:, 128 * t0:128 * (t0 + nt)])
                        for s_i in range(nt):
                            t = t0 + s_i
                            b = tiles[t][1]
                            first = runs[b][w][0] == t
                            last = runs[b][w][1] == t
                            preload = w == 1 and runs[b][0] is not None
                            if first:
                                cur_ps = pbps.tile([128, 128], _f32, tag="blk")
                                if preload and "mm" not in SKIP:
                                    # seed the accumulation with the window-0
                                    # partial so no separate add is needed
                                    nc.scalar.copy(cur_ps[:], acc_sb[:, b, :])
                                if "mm" in SKIP:
                                    nc.vector.memset(cur_ps[:], 0.0)
                            if "mm" not in SKIP:
                                nc.tensor.matmul(
                                    cur_ps[:],
                                    lhsT=st_sb[:, 128 * s_i:128 * (s_i + 1)],
                                    rhs=gbuf[:, s_i, :],
                                    start=first and not preload, stop=last)
                            if last:
                                nc.scalar.copy(acc_sb[:, b, :], cur_ps[:])
                                cur_ps = None
                        t0 += nt
                        nt_left -= nt

                with tc.tile_pool(name="p1x", bufs=2) as p1x, \
                     tc.tile_pool(name="p1h", bufs=3) as p1h, \
                     tc.tile_pool(name="p1ps", bufs=3, space="PSUM") as p1ps, \
                     tc.tile_pool(name="stp", bufs=3) as stp, \
                     tc.tile_pool(name="gb", bufs=GBUFS) as gbp, \
                     tc.tile_pool(name="pbps", bufs=4, space="PSUM") as pbps, \
                     tc.tile_pool(name="ep", bufs=2) as ep:
                    # p1(0), edge(0), p1(1), edge(1): keeps PE's in-order
                    # queue from parking edge-A matmuls behind P1-B, which
                    # would stall gather-A once the gather buffers fill
                    p1_window(0)
                    edge_window(0)
                    p1_window(1)
                    edge_window(1)
                    # bias_gated is written after the last edge-pass psum
                    # copy in ACT program order; every epilogue reads it, so
                    # no epilogue DVE op can start before the gathers end
                    bias_gated = ep.tile([128, 128], _f32, tag="e_bg")
                    nc.scalar.copy(bias_gated[:], bias_bc[:])
                    for b in range(NB):
                        epilogue(b, bias_gated)

            if reps == 1:
                _phases()
            else:
                with tc.For_i(0, reps, 1):
                    _phases()

    nc.compile()
    return nc


def _core_maps(x, weight, bias, gamma, beta, st, idx16):
    """Per-core input maps (shared by kernel() and test.py)."""
    xt = np.zeros((128, SRC_PAD), ml_dtypes.bfloat16)
    xt[:, :N] = np.asarray(x, np.float32).T.astype(ml_dtypes.bfloat16)
    wb = np.asarray(weight, np.float32).astype(ml_dtypes.bfloat16)
    bias = np.asarray(bias, np.float32).reshape(1, D)
    gamma = np.asarray(gamma, np.float32).reshape(1, D)
    beta = np.asarray(beta, np.float32).reshape(1, D)
    maps = []
    for c in range(C):
        maps.append({
            "xt": xt, "w": wb, "bias": bias, "gamma": gamma, "beta": beta,
            "st": np.ascontiguousarray(st[c]),
            "idx16": np.ascontiguousarray(idx16[c]),
        })
    return maps


_CACHE = {}
_OUTMAP = None


def _get_compiled(edge_index, edge_weight):
    P, T, st, idx16 = _preprocess(edge_index, edge_weight)
    key = P.tobytes()
    if key not in _CACHE:
        tiles, t0w, Tw = _schedule(P)
        _CACHE[key] = _build(P, T, tiles, t0w, Tw)
    return _CACHE[key], st, idx16


def kernel(x, edge_index, edge_weight, weight, adaptive_weight, bias,
           ln_gamma, ln_beta):
    nc, st, idx16 = _get_compiled(edge_index, edge_weight)
    in_maps = _core_maps(x, weight, bias, ln_gamma, ln_beta, st, idx16)
    res = run_bass_kernel_spmd(nc, in_maps, core_ids=list(range(C)))
    out = np.empty((N, D), np.float32)
    for c in range(C):
        oc = np.asarray(res.results[c]["out"])
        m = _OUTMAP[c]
        valid = m >= 0
        out[c * NPC + m[valid]] = oc[valid]
    return out
